# revision 29
# baseline (speedup 1.0000x reference)
"""MultiHeadLatentAttn TRN2 kernel (8 NeuronCores, uniform SPMD). v4.

fp8-e4m3 DoubleRow matmuls throughout: down-proj, pos_k, q/k/v/pos_q,
scores, attn@v, denominator, o_proj. Rows 0..255 are recomputed
exactly on the host (fp32): they attend over too few keys for fp8
noise to average out.

v4 removes the latent AllGather: the CC stream costs 80-150us of
boot/barrier/gather latency with high core-to-core variance, more
than the 4x-replicated down-projection it saved (+288 cheap fp8-DR
matmuls/core). Each core computes the FULL latent of its batch
(2048 tokens) chunk by chunk, interleaved with B1 (q/k/v for its 4
heads) and B2 (attention + o_proj for query block qB=chunk), so the
PE stream is dense from ~4us with no cross-core dependency at all.
qc8/kc8 packing: [128, 2*S]; plane i=0 = 128 main dims, plane i=1 =
64 rope'd pos dims + 64 zero rows; scores = one DR matmul per
128-key tile. Host: sums 4 partial OT per batch, adds b_o, patches
rows 0..255.
"""

import os
import sys

import numpy as np

for _p in ("/opt/trn_rl_repo", "/root/.axon_site/_ro/trn_rl_repo"):
    if os.path.isdir(_p) and _p not in sys.path:
        sys.path.append(_p)

import concourse.bass as bass
import concourse.mybir as mybir
import concourse.tile as tile
from concourse import bacc
from concourse import bass_utils

F32 = mybir.dt.float32
BF16 = mybir.dt.bfloat16
FP8 = mybir.dt.float8e4
DR = mybir.MatmulPerfMode.DoubleRow

MODEL = 2048
LATENT = 512
L3 = 3 * LATENT            # 1536
NH = 16
HD = 128
PHD = 64
DC = HD + PHD              # 192
B, S = 2, 2048
TOK = 512                  # tokens per core in phase A
NCORES = 8
ROPE_THETA = 50000.0
SCALE = 1.0 / float(np.sqrt(DC))
HOST_ROWS = 256            # rows recomputed exactly on host

RG = [[0, 1, 2, 3], [4, 5, 6, 7]]
F32_INPUTS = {"bd", "bk", "bq", "bqp", "bkp"}
FP8_INPUTS = {"x8", "Wd8", "Wkp8", "Wuq8", "Wuk8", "Wuv8", "Wqp8",
              "WoS8"}


def _emit(nc, tc, T):
    from contextlib import ExitStack
    Ex = mybir.ActivationFunctionType.Exp
    Ident = mybir.ActivationFunctionType.Identity

    def r2(ap):
        return ap.rearrange("p (i t) -> p i t", i=2)

    if True:
        persist_ctx = ExitStack()
        persistp = persist_ctx.enter_context(
            tc.tile_pool(name="persist", bufs=1))
        # packed fp8 score operands: cols [0,S) main dims (128 rows);
        # cols [S,2S): rows 0:64 rope'd pos dims, rows 64:128 zeros
        qc8 = [persistp.tile([128, 2 * S], FP8, name=f"qc8{h}",
                             tag=f"qc{h}") for h in range(4)]
        kc8 = [persistp.tile([128, 2 * S], FP8, name=f"kc8{h}",
                             tag=f"kc{h}") for h in range(4)]
        # v pairs: vt8[m] cols [0,512) = token-tile 2m, [512,1024) = 2m+1
        vt8 = [persistp.tile([128, 1024], FP8, name=f"vt8{m}",
                             tag=f"vt{m}") for m in range(8)]
        # attn pairs for fp8 o_proj: attn8[u] plane i = head 2u+i
        attn8 = [persistp.tile([128, 2 * S], FP8, name=f"attn8{u}",
                               tag=f"at{u}") for u in range(2)]
        latg8 = [persistp.tile([128, 2 * S], FP8, name=f"latg8{j}",
                               tag=f"lg{j}") for j in range(6)]
        wos8 = persistp.tile([128, 8192], FP8, name="wos8", tag="wos8")
        maskp = [persistp.tile([128, 1024], BF16, name=f"maskp{m}",
                               tag=f"mp{m}") for m in range(2)]
        cosq = persistp.tile([128, S], BF16, name="cosq", tag="cosq")
        sinq = persistp.tile([128, S], BF16, name="sinq", tag="sinq")
        bk = persistp.tile([128, 4], F32, name="bk", tag="bk")
        bq = persistp.tile([128, 4], F32, name="bq", tag="bq")
        bqp = persistp.tile([128, 2], F32, name="bqp", tag="bqp")
        bv = persistp.tile([1, 512], BF16, name="bv", tag="bv")
        ones1 = persistp.tile([1, 128], BF16, name="ones1", tag="ones1")
        ONES8 = persistp.tile([128, 256], FP8, name="ONES8", tag="ONES8")
        dume = persistp.tile([128, 1], BF16, name="dume", tag="dume")
        nc.vector.memset(ONES8[:], 1.0)
        nc.vector.memset(ones1[:], 1.0)
        nc.vector.memset(dume[:], 0.0)
        # zero the pos-padding rows once (fp8 zeros contribute nothing)
        for h in range(4):
            nc.vector.memset(qc8[h][64:128, S:2 * S], 0.0)
            nc.vector.memset(kc8[h][64:128, S:2 * S], 0.0)

        # B1 weights: loaded during phase A on the scalar ring
        wu_ctx = ExitStack()
        wup = wu_ctx.enter_context(tc.tile_pool(name="wu", bufs=1))
        wuq8 = wup.tile([128, 6144], FP8, name="wuq8", tag="wuq8")
        wuk8 = wup.tile([128, 6144], FP8, name="wuk8", tag="wuk8")
        wuv8 = wup.tile([128, 6144], FP8, name="wuv8", tag="wuv8")
        wqp8 = wup.tile([128, 1024], FP8, name="wqp8", tag="wqp8")

        # ------------- inputs (loaded once, used by the tc loop) -------
        inp_ctx = ExitStack()
        xap = inp_ctx.enter_context(tc.tile_pool(name="xa", bufs=1))
        cA = inp_ctx.enter_context(tc.tile_pool(name="cA", bufs=1))
        x8t = [xap.tile([128, 8192], FP8, name=f"x8t{tc}", tag=f"x8{tc}")
               for tc in range(4)]
        wd = [[xap.tile([128, 4096], FP8, name=f"wd{g}{hh}",
                        tag=f"wd{g}{hh}") for hh in range(2)]
              for g in range(3)]
        wkp8 = cA.tile([128, 2048], FP8, name="wkp8", tag="wkp8")
        bd = cA.tile([128, 12], F32, name="bd", tag="bd")
        bkp = cA.tile([PHD, 1], F32, name="bkp", tag="bkp")
        cosk = cA.tile([PHD, S], BF16, name="cosk", tag="cosk")
        sink = cA.tile([PHD, S], BF16, name="sink", tag="sink")

        # sync ring: chunk-0 x + half the g0 weights first (unblocks
        # the first matmuls), then consts, then the later x chunks
        nc.sync.dma_start(x8t[0][:], T["x8"][:, 0:8192])
        nc.sync.dma_start(wd[0][1][:], T["Wd8"][:, 4096:8192])
        nc.sync.dma_start(bd[:], T["bd"][:])
        nc.sync.dma_start(bkp[:], T["bkp"][:])
        nc.sync.dma_start(cosk[:], T["cosk"][:])
        nc.sync.dma_start(sink[:], T["sink"][:])
        for tc_ in range(1, 4):
            nc.sync.dma_start(x8t[tc_][:],
                              T["x8"][:, tc_ * 8192:(tc_ + 1) * 8192])
        # scalar ring: Wd halves first, then B1 weights
        for g, hh in ((0, 0), (1, 0), (1, 1), (2, 0), (2, 1)):
            nc.scalar.dma_start(
                wd[g][hh][:],
                T["Wd8"][:, g * 8192 + hh * 4096:
                         g * 8192 + (hh + 1) * 4096])
        nc.scalar.dma_start(wkp8[:], T["Wkp8"][:])
        nc.scalar.dma_start(wuq8[:], T["Wuq8"][:])
        nc.scalar.dma_start(wuk8[:], T["Wuk8"][:])
        nc.scalar.dma_start(wuv8[:], T["Wuv8"][:])
        nc.scalar.dma_start(wqp8[:], T["Wqp8"][:])
        nc.scalar.dma_start(bq[:], T["bq"][:])
        nc.scalar.dma_start(bk[:], T["bk"][:])
        nc.scalar.dma_start(bqp[:], T["bqp"][:])
        nc.scalar.dma_start(bv[:], T["bv"][:])
        # preload exp table while the first matmuls run
        nc.scalar.activation(dume[:], dume[:], Ex)
        # B2-only weights on the gpsimd ring
        nc.gpsimd.dma_start(wos8[:], T["WoS8"][:])
        nc.gpsimd.dma_start(maskp[0][:], T["maskp0"][:])
        nc.gpsimd.dma_start(maskp[1][:], T["maskp1"][:])
        nc.gpsimd.dma_start(cosq[:], T["cosq"][:])
        nc.gpsimd.dma_start(sinq[:], T["sinq"][:])

        # ------------- A(tc) + B1(c=tc) + B2(qB=tc) loop -------------
        with (
            tc.tile_pool(name="ropet", bufs=1) as ropet,
            tc.tile_pool(name="ep", bufs=6) as ep,
            tc.tile_pool(name="ebp", bufs=2) as ebp,
            tc.tile_pool(name="rcp", bufs=3) as rcp,
            tc.tile_pool(name="oep", bufs=4) as oep,
            tc.tile_pool(name="psB", bufs=1, space="PSUM") as psB,
        ):
            def pt(tag, name, rows=128):
                return psB.tile([rows, 512], F32, name=name, tag=tag)

            def xw(tc_, w):
                return r2(x8t[tc_][:, w * 1024:(w + 1) * 1024])

            for c in range(4):
                cs = slice(c * 512, (c + 1) * 512)
                # ---- A: down-proj latent for tokens chunk c ----
                for g in range(3):
                    pa = [pt(f"b{i}", f"pa{c}{g}{i}") for i in range(4)]
                    for w in range(8):
                        lhs = r2(wd[g][w // 4][:, (w % 4) * 1024:
                                              (w % 4 + 1) * 1024])
                        for i in range(4):
                            nc.tensor.matmul(
                                pa[i][:], lhs[:, :, i * 128:(i + 1) * 128],
                                xw(c, w), start=(w == 0), stop=(w == 7),
                                perf_mode=DR, skip_group_check=True)
                    for i in range(4):
                        lt = g * 4 + i
                        nc.scalar.activation(
                            latg8[lt // 2][:, (lt % 2) * S + c * 512:
                                           (lt % 2) * S + (c + 1) * 512],
                            pa[i][:], Ident, bias=bd[:, lt:lt + 1])
                # ---- pos_k for tokens chunk c (padded to M=128) ----
                psk = pt("b4", f"psk{c}")
                for w in range(8):
                    nc.tensor.matmul(
                        psk[:], r2(wkp8[:, w * 256:(w + 1) * 256]),
                        xw(c, w), start=(w == 0), stop=(w == 7),
                        perf_mode=DR, skip_group_check=True)
                pkraw = ropet.tile([PHD, 512], BF16, name=f"pkraw{c}",
                                   tag="pkraw")
                nc.scalar.activation(pkraw[:], psk[0:64, :], Ident,
                                     bias=bkp[:, 0:1])
                pk1 = ropet.tile([PHD, 512], BF16, name=f"pk1{c}", tag="pk1")
                pku = ropet.tile([PHD, 512], BF16, name=f"pku{c}", tag="pku")
                pkr = ropet.tile([PHD, 512], BF16, name=f"pkr{c}", tag="pkr")
                pk8f = ropet.tile([PHD, 512], FP8, name=f"pk8f{c}",
                                  tag="pk8f")
                nc.vector.tensor_mul(pk1[:], pkraw[:], cosk[:, cs])
                nc.vector.tensor_mul(pku[:], pkraw[:], sink[:, cs])
                nc.sync.dma_start(pkr[0:32, :], pku[32:64, :])
                nc.sync.dma_start(pkr[32:64, :], pku[0:32, :])
                nc.vector.tensor_add(pk8f[:], pk1[:], pkr[:])
                for h in range(4):
                    nc.vector.tensor_copy(
                        kc8[h][0:64, S + c * 512:S + (c + 1) * 512],
                        pk8f[:])
                # B2 for the previous query block, emitted AFTER this
                # chunk's dependency-free A matmuls so they fill the
                # rope->qc8 latency bubble left by wave2(c-1)
                if c >= 1:
                    emit_b2(c - 1)
                # ---- wave 1: q (4 heads) + k (4 heads) -> 8 banks ----
                psq = [pt(f"b{h}", f"psq{c}{h}") for h in range(4)]
                psk_ = [pt(f"b{4 + d}", f"psk{c}{d}") for d in range(4)]
                for j in range(6):
                    lat_r = r2(latg8[j][:])[:, :, cs]
                    wq = r2(wuq8[:, j * 1024:(j + 1) * 1024])
                    wk = r2(wuk8[:, j * 1024:(j + 1) * 1024])
                    for h in range(4):
                        nc.tensor.matmul(
                            psq[h][:], wq[:, :, h * 128:(h + 1) * 128],
                            lat_r, start=(j == 0), stop=(j == 5),
                            perf_mode=DR, skip_group_check=True)
                    for d in range(4):
                        nc.tensor.matmul(
                            psk_[d][:], wk[:, :, d * 128:(d + 1) * 128],
                            lat_r, start=(j == 0), stop=(j == 5),
                            perf_mode=DR, skip_group_check=True)
                for h in range(4):
                    nc.scalar.activation(qc8[h][:, cs], psq[h][:], Ident,
                                         bias=bq[:, h:h + 1])
                for d in range(4):
                    nc.scalar.activation(kc8[d][:, cs], psk_[d][:], Ident,
                                         bias=bk[:, d:d + 1])
                # ---- wave 2: v (4 token-tiles) + pos_q (2) -> 6 banks ----
                psv = [pt(f"b{i}", f"psv{c}{i}") for i in range(4)]
                pspq = [pt(f"b{4 + p_}", f"pspq{c}{p_}") for p_ in range(2)]
                for j in range(6):
                    lat_r = r2(latg8[j][:])
                    wv = r2(wuv8[:, j * 1024:(j + 1) * 1024])
                    for jt in range(4):
                        tcol = c * 512 + jt * 128
                        nc.tensor.matmul(
                            psv[jt][:], lat_r[:, :, tcol:tcol + 128],
                            wv, start=(j == 0), stop=False,
                            perf_mode=DR, skip_group_check=True)
                    if j < 2:
                        wp = r2(wqp8[:, j * 512:(j + 1) * 512])
                        for p_ in range(2):
                            nc.tensor.matmul(
                                pspq[p_][:],
                                wp[:, :, p_ * 128:(p_ + 1) * 128],
                                r2(latg8[j][:])[:, :, cs],
                                start=(j == 0), stop=(j == 1),
                                perf_mode=DR, skip_group_check=True)
                for jt in range(4):
                    nc.tensor.matmul(psv[jt][:], ones1[:], bv[:],
                                     start=False, stop=True)
                    tt = c * 4 + jt
                    nc.vector.tensor_copy(
                        vt8[tt // 2][:, (tt % 2) * 512:(tt % 2 + 1) * 512],
                        psv[jt][:])
                for p_ in range(2):
                    raw = ropet.tile([128, 512], BF16, name=f"pqr{c}{p_}",
                                     tag="praw")
                    nc.scalar.activation(raw[:], pspq[p_][:], Ident,
                                         bias=bqp[:, p_:p_ + 1])
                    t1 = ropet.tile([128, 512], BF16, name=f"t1{c}{p_}",
                                    tag="t1")
                    tu = ropet.tile([128, 512], BF16, name=f"tu{c}{p_}",
                                    tag="tu")
                    tr = ropet.tile([128, 512], BF16, name=f"tr{c}{p_}",
                                    tag="tr")
                    nc.vector.tensor_mul(t1[:], raw[:], cosq[:, cs])
                    nc.vector.tensor_mul(tu[:], raw[:], sinq[:, cs])
                    for h2 in range(2):
                        o = h2 * 64
                        nc.sync.dma_start(tr[o:o + 32, :],
                                          tu[o + 32:o + 64, :])
                        nc.sync.dma_start(tr[o + 32:o + 64, :],
                                          tu[o:o + 32, :])
                    nc.vector.tensor_add(t1[:], t1[:], tr[:])
                    # bf16 -> fp8 cast DMA (SWDGE) into the pos plane
                    nc.gpsimd.dma_start(qc8[2 * p_][0:64, S + c * 512:
                                                    S + (c + 1) * 512],
                                        t1[0:64, :])
                    nc.gpsimd.dma_start(qc8[2 * p_ + 1][0:64, S + c * 512:
                                                        S + (c + 1) * 512],
                                        t1[64:128, :])

                # wave1/wave2 for chunk c emitted below; B2 for qB=c is
                # deferred one iteration so A(c+1)'s dependency-free
                # matmuls fill the rope->qc8 latency bubble
                emit_b2(c)

            def emit_b2(qB):
                qs = slice(qB * 512, (qB + 1) * 512)
                npair = 2 * qB + 2
                for h in range(4):
                    av = pt("b4", f"av{h}{qB}")
                    den = pt("b5", f"dn{h}{qB}")
                    ee = []

                    def scores(p):
                        e8 = ep.tile([128, 1024], FP8, name=f"e{h}{qB}{p}",
                                     tag="e")
                        diag = (p - 2 * qB) >= 0
                        eb = None
                        if diag:
                            eb = ebp.tile([128, 1024], BF16,
                                          name=f"eb{h}{qB}{p}", tag="eb")
                        for jj in range(2):
                            kt = 2 * p + jj
                            ks = slice(kt * 128, (kt + 1) * 128)
                            sc_t = pt(f"b{jj}" if p % 2 == 0 else
                                      f"b{2 + jj}", f"s{h}{qB}{p}{jj}")
                            nc.tensor.matmul(
                                sc_t[:], r2(kc8[h][:])[:, :, ks],
                                r2(qc8[h][:])[:, :, qs],
                                start=True, stop=True,
                                perf_mode=DR, skip_group_check=True)
                            js = slice(jj * 512, (jj + 1) * 512)
                            if diag:
                                nc.scalar.activation(eb[:, js], sc_t[:],
                                                     Ex, scale=SCALE)
                            else:
                                nc.scalar.activation(e8[:, js], sc_t[:],
                                                     Ex, scale=SCALE)
                        if diag:
                            nc.vector.tensor_mul(e8[:], eb[:],
                                                 maskp[p - 2 * qB][:])
                        ee.append(e8)

                    def accum(p):
                        e8r = r2(ee[p][:])
                        nc.tensor.matmul(
                            den[:], r2(ONES8[:]), e8r,
                            start=(p == 0), stop=(p == npair - 1),
                            perf_mode=DR, skip_group_check=True)
                        nc.tensor.matmul(
                            av[:],
                            r2(vt8[p][:])[:, :, h * 128:(h + 1) * 128],
                            e8r, start=(p == 0), stop=(p == npair - 1),
                            perf_mode=DR, skip_group_check=True)

                    scores(0)
                    if npair > 1:
                        scores(1)
                    for p in range(2, npair):
                        scores(p)
                        accum(p - 2)
                    if npair > 1:
                        accum(npair - 2)
                    accum(npair - 1)
                    rc = rcp.tile([128, 512], F32, name=f"rc{h}{qB}",
                                  tag="rc")
                    nc.vector.reciprocal_approx_fast(rc[:], den[:])
                    nc.vector.tensor_mul(
                        attn8[h // 2][:, (h % 2) * S + qB * 512:
                                      (h % 2) * S + (qB + 1) * 512],
                        av[:], rc[:])

                # o_proj (fp8 DR) for this query block
                for mt in range(16):
                    op = pt("b6" if mt % 2 == 0 else "b7", f"op{qB}{mt}")
                    for u in range(2):
                        nc.tensor.matmul(
                            op[:],
                            r2(wos8[:, u * 4096:(u + 1) * 4096])[
                                :, :, mt * 128:(mt + 1) * 128],
                            r2(attn8[u][:])[:, :, qs],
                            start=(u == 0), stop=(u == 1),
                            perf_mode=DR, skip_group_check=True)
                    oe = oep.tile([128, 512], BF16, name=f"oe{qB}{mt}",
                                  tag="oe")
                    nc.vector.tensor_copy(oe[:], op[:])
                    eng = nc.gpsimd if mt % 2 == 0 else nc.sync
                    eng.dma_start(T["OT"][mt * 128:(mt + 1) * 128, qs],
                                  oe[:])

        inp_ctx.close()
        wu_ctx.close()
        persist_ctx.close()


def build_program():
    nc = bacc.Bacc("TRN2", target_bir_lowering=False, debug=False,
                   num_devices=NCORES)
    T = {}

    def inp(name, shape, dt=BF16):
        T[name] = nc.dram_tensor(name, shape, dt, kind="ExternalInput").ap()

    inp("x8", [128, 32768], FP8)
    inp("Wd8", [128, 24576], FP8)
    inp("Wkp8", [128, 2048], FP8)
    inp("Wuq8", [128, 6144], FP8)
    inp("Wuk8", [128, 6144], FP8)
    inp("Wuv8", [128, 6144], FP8)
    inp("Wqp8", [128, 1024], FP8)
    inp("WoS8", [128, 8192], FP8)
    inp("cosq", [128, S])
    inp("sinq", [128, S])
    inp("cosk", [PHD, S])
    inp("sink", [PHD, S])
    inp("bd", [128, 12], F32)
    inp("bk", [128, 4], F32)
    inp("bq", [128, 4], F32)
    inp("bqp", [128, 2], F32)
    inp("bkp", [PHD, 1], F32)
    inp("bv", [1, 512])
    inp("maskp0", [128, 1024])
    inp("maskp1", [128, 1024])
    T["OT"] = nc.dram_tensor("OT", [MODEL, S], BF16,
                             kind="ExternalOutput").ap()

    with tile.TileContext(nc) as tc:
        _emit(nc, tc, T)
    nc.compile()
    return nc


def host_inputs(inputs):
    import ml_dtypes
    bf16 = ml_dtypes.bfloat16
    f8 = ml_dtypes.float8_e4m3
    x = np.ascontiguousarray(np.asarray(inputs["x"], np.float32))
    W_down = np.asarray(inputs["W_down"], np.float32)
    b_down = np.asarray(inputs["b_down"], np.float32)
    W_up = np.asarray(inputs["W_up"], np.float32)
    b_up = np.asarray(inputs["b_up"], np.float32)
    W_qpos = np.asarray(inputs["W_qpos"], np.float32)
    b_qpos = np.asarray(inputs["b_qpos"], np.float32)
    W_kpos = np.asarray(inputs["W_kpos"], np.float32)
    b_kpos = np.asarray(inputs["b_kpos"], np.float32)
    W_o = np.asarray(inputs["W_o"], np.float32)

    inv = (1.0 / ROPE_THETA ** (np.arange(0, PHD, 2, dtype=np.float32) / PHD))
    t_all = np.arange(S, dtype=np.float32)
    fr = np.outer(inv, t_all)                       # [32, S]
    cc = np.concatenate([np.cos(fr), np.cos(fr)], 0)        # [64, S]
    ss = np.sin(fr)
    ssn = np.concatenate([ss, -ss], 0)                      # [64, S]
    cosq = np.vstack([cc, cc]).astype(np.float32)           # [128, S]
    sinq = np.vstack([ssn, ssn]).astype(np.float32)

    qq = np.arange(512)[None, :]
    kk = np.arange(128)[:, None]
    masks = [np.where(qq >= kk + m * 128, 1.0, 0.0).astype(np.float32)
             for m in range(4)]
    maskp0 = np.ascontiguousarray(np.concatenate([masks[0], masks[1]], 1))
    maskp1 = np.ascontiguousarray(np.concatenate([masks[2], masks[3]], 1))

    def pack_pairs(w, npair_, inner):
        # [npair_*2*128, inner] -> [128, npair_*2*inner] cols (j, i, f)
        return np.ascontiguousarray(
            w.reshape(npair_, 2, 128, inner).transpose(2, 0, 1, 3).reshape(
                128, npair_ * 2 * inner))

    # Wd8: cols g*8192 + w*1024 + i*512 + fg
    Wd8 = np.ascontiguousarray(
        W_down.reshape(8, 2, 128, 3, 512).transpose(2, 3, 0, 1, 4).reshape(
            128, 24576))

    common = {
        "Wd8": Wd8,
        "Wkp8": pack_pairs(
            np.concatenate([W_kpos, np.zeros((MODEL, PHD), np.float32)],
                           1), 8, 2 * PHD),
        "cosk": cc, "sink": ssn,
        "cosq": cosq, "sinq": sinq,
        "bd": np.ascontiguousarray(b_down.reshape(12, 128).T),
        "bkp": np.ascontiguousarray(b_kpos[:, None]),
        "maskp0": maskp0, "maskp1": maskp1,
    }
    maps = []
    for c in range(NCORES):
        b, j = divmod(c, 4)
        ts = slice(j * TOK, (j + 1) * TOK)
        hs = slice(j * 512, (j + 1) * 512)
        m = dict(common)
        # x8: full batch, cols tc*8192 + w*1024 + i*512 + t
        xT = np.ascontiguousarray(x[b].T)                   # [2048, 2048]
        m["x8"] = np.ascontiguousarray(
            xT.reshape(8, 2, 128, 4, 512).transpose(2, 3, 0, 1, 4)
            .reshape(128, 32768))
        m["Wuq8"] = pack_pairs(W_up[:, :MODEL][:, hs], 6, 512)
        m["Wuk8"] = pack_pairs(W_up[:, MODEL:2 * MODEL][:, hs], 6, 512)
        m["Wuv8"] = pack_pairs(W_up[:, 2 * MODEL:][:, hs], 6, 512)
        m["Wqp8"] = pack_pairs(
            np.ascontiguousarray(W_qpos[:, j * 256:(j + 1) * 256]), 2, 256)
        m["WoS8"] = np.ascontiguousarray(
            W_o[hs, :].reshape(2, 2, 128, MODEL).transpose(
                2, 0, 1, 3).reshape(128, 8192))
        m["bq"] = np.ascontiguousarray(b_up[:MODEL][hs].reshape(4, 128).T)
        m["bk"] = np.ascontiguousarray(
            b_up[MODEL:2 * MODEL][hs].reshape(4, 128).T)
        m["bqp"] = np.ascontiguousarray(
            b_qpos[j * 256:(j + 1) * 256].reshape(2, 128).T)
        m["bv"] = np.ascontiguousarray(b_up[2 * MODEL:][hs][None, :])
        for key in list(m):
            if key in F32_INPUTS:
                m[key] = np.ascontiguousarray(m[key], np.float32)
            elif key in FP8_INPUTS:
                m[key] = np.ascontiguousarray(m[key]).astype(f8)
            else:
                m[key] = np.ascontiguousarray(m[key]).astype(bf16)
        maps.append(m)
    return maps


def _host_head(inputs, R):
    # exact fp32 recompute of output rows [0, R) (causal: needs only
    # the first R tokens)
    x = np.asarray(inputs["x"], np.float32)[:, :R, :]
    W_down = np.asarray(inputs["W_down"], np.float32)
    b_down = np.asarray(inputs["b_down"], np.float32)
    W_up = np.asarray(inputs["W_up"], np.float32)
    b_up = np.asarray(inputs["b_up"], np.float32)
    W_qpos = np.asarray(inputs["W_qpos"], np.float32)
    b_qpos = np.asarray(inputs["b_qpos"], np.float32)
    W_kpos = np.asarray(inputs["W_kpos"], np.float32)
    b_kpos = np.asarray(inputs["b_kpos"], np.float32)
    W_o = np.asarray(inputs["W_o"], np.float32)
    b_o = np.asarray(inputs["b_o"], np.float32)

    Bn, Sn, M = x.shape
    lat = x @ W_down + b_down
    fused = lat @ W_up + b_up
    q, k, v = np.split(fused, 3, axis=-1)

    def to_heads(t, nh):
        return t.reshape(Bn, Sn, nh, -1).transpose(0, 2, 1, 3)

    q, k, v = to_heads(q, NH), to_heads(k, NH), to_heads(v, NH)
    pos_q = to_heads(lat[..., :LATENT] @ W_qpos + b_qpos, NH)
    pos_k = to_heads(x @ W_kpos + b_kpos, 1)

    inv = 1.0 / ROPE_THETA ** (np.arange(0, PHD, 2, dtype=np.float32) / PHD)
    t_ = np.arange(Sn, dtype=np.float32)
    fre = np.outer(t_, inv)
    cos = np.concatenate([np.cos(fre), np.cos(fre)], -1)[None, None]
    sin = np.concatenate([np.sin(fre), np.sin(fre)], -1)[None, None]

    def rot(p):
        return np.concatenate([-p[..., PHD // 2:], p[..., :PHD // 2]], -1)

    pos_q = pos_q * cos + rot(pos_q) * sin
    pos_k = pos_k * cos + rot(pos_k) * sin
    pos_k = np.broadcast_to(pos_k, (Bn, NH, Sn, PHD))
    qc = np.concatenate([q, pos_q], -1)
    kc = np.concatenate([k, pos_k], -1)
    sc = np.einsum("bhsd,bhtd->bhst", qc, kc) * np.float32(SCALE)
    causal = np.tril(np.ones((Sn, Sn), bool))
    sc = np.where(causal[None, None], sc, np.float32(-1e30))
    sc = sc - sc.max(-1, keepdims=True)
    p = np.exp(sc)
    p /= p.sum(-1, keepdims=True)
    at = np.einsum("bhst,bhtd->bhsd", p, v)
    at = at.transpose(0, 2, 1, 3).reshape(Bn, Sn, M)
    return at @ W_o + b_o


_NC_CACHE = None


def _program():
    global _NC_CACHE
    if _NC_CACHE is None:
        _NC_CACHE = build_program()
    return _NC_CACHE


def kernel(**inputs) -> np.ndarray:
    nc = _program()
    maps = host_inputs(inputs)
    kwargs = {}
    if os.environ.get("BASSK_TRACE"):
        kwargs = dict(trace=True, trace_cores=list(range(NCORES)))
        td = os.environ.get("BASSK_TRACE_DIR")
        if td:
            kwargs["tmpdir"] = td
    res = bass_utils.run_bass_kernel_spmd(
        nc, maps, core_ids=list(range(NCORES)), **kwargs)
    kernel.last_results = res
    b_o = np.asarray(inputs["b_o"], np.float32)
    out = np.empty((B, S, MODEL), np.float32)
    for b in range(B):
        acc = np.asarray(res.results[b * 4]["OT"], np.float32)
        for c in range(b * 4 + 1, b * 4 + 4):
            acc = acc + np.asarray(res.results[c]["OT"], np.float32)
        out[b] = acc.T + b_o[None, :]
    out[:, :HOST_ROWS, :] = _host_head(inputs, HOST_ROWS)
    return out


# revision 30
# speedup vs baseline: 1.1718x; 1.1718x over previous
"""MultiHeadLatentAttn TRN2 kernel (8 NeuronCores, uniform SPMD). v4.

fp8-e4m3 DoubleRow matmuls throughout: down-proj, pos_k, q/k/v/pos_q,
scores, attn@v, denominator, o_proj. Rows 0..255 are recomputed
exactly on the host (fp32): they attend over too few keys for fp8
noise to average out.

v4 removes the latent AllGather: the CC stream costs 80-150us of
boot/barrier/gather latency with high core-to-core variance, more
than the 4x-replicated down-projection it saved (+288 cheap fp8-DR
matmuls/core). Each core computes the FULL latent of its batch
(2048 tokens) chunk by chunk, interleaved with B1 (q/k/v for its 4
heads) and B2 (attention + o_proj for query block qB=chunk), so the
PE stream is dense from ~4us with no cross-core dependency at all.
qc8/kc8 packing: [128, 2*S]; plane i=0 = 128 main dims, plane i=1 =
64 rope'd pos dims + 64 zero rows; scores = one DR matmul per
128-key tile. Host: sums 4 partial OT per batch, adds b_o, patches
rows 0..255.
"""

import os
import sys

import numpy as np

for _p in ("/opt/trn_rl_repo", "/root/.axon_site/_ro/trn_rl_repo"):
    if os.path.isdir(_p) and _p not in sys.path:
        sys.path.append(_p)

import concourse.bass as bass
import concourse.mybir as mybir
import concourse.tile as tile
from concourse import bacc
from concourse import bass_utils

F32 = mybir.dt.float32
BF16 = mybir.dt.bfloat16
FP8 = mybir.dt.float8e4
DR = mybir.MatmulPerfMode.DoubleRow

MODEL = 2048
LATENT = 512
L3 = 3 * LATENT            # 1536
NH = 16
HD = 128
PHD = 64
DC = HD + PHD              # 192
B, S = 2, 2048
TOK = 512                  # tokens per core in phase A
NCORES = 8
ROPE_THETA = 50000.0
SCALE = 1.0 / float(np.sqrt(DC))
HOST_ROWS = 256            # rows recomputed exactly on host

RG = [[0, 1, 2, 3], [4, 5, 6, 7]]
F32_INPUTS = {"bd", "bk", "bq", "bqp", "bkp"}
FP8_INPUTS = {"x8", "Wd8", "Wkp8", "Wuq8", "Wuk8", "Wuv8", "Wqp8",
              "WoS8"}


def _emit(nc, tc, T):
    from contextlib import ExitStack
    Ex = mybir.ActivationFunctionType.Exp
    Ident = mybir.ActivationFunctionType.Identity

    def r2(ap):
        return ap.rearrange("p (i t) -> p i t", i=2)

    if True:
        persist_ctx = ExitStack()
        persistp = persist_ctx.enter_context(
            tc.tile_pool(name="persist", bufs=1))
        # packed fp8 score operands: cols [0,S) main dims (128 rows);
        # cols [S,2S): rows 0:64 rope'd pos dims, rows 64:128 zeros
        qc8 = [persistp.tile([128, 2 * S], FP8, name=f"qc8{h}",
                             tag=f"qc{h}") for h in range(4)]
        kc8 = [persistp.tile([128, 2 * S], FP8, name=f"kc8{h}",
                             tag=f"kc{h}") for h in range(4)]
        # v pairs: vt8[m] cols [0,512) = token-tile 2m, [512,1024) = 2m+1
        vt8 = [persistp.tile([128, 1024], FP8, name=f"vt8{m}",
                             tag=f"vt{m}") for m in range(8)]
        # attn pairs for fp8 o_proj: attn8[u] plane i = head 2u+i
        attn8 = [persistp.tile([128, 2 * S], FP8, name=f"attn8{u}",
                               tag=f"at{u}") for u in range(2)]
        latg8 = [persistp.tile([128, 2 * S], FP8, name=f"latg8{j}",
                               tag=f"lg{j}") for j in range(6)]
        wos8 = persistp.tile([128, 8192], FP8, name="wos8", tag="wos8")
        maskp = [persistp.tile([128, 1024], BF16, name=f"maskp{m}",
                               tag=f"mp{m}") for m in range(2)]
        cosq = persistp.tile([128, S], BF16, name="cosq", tag="cosq")
        sinq = persistp.tile([128, S], BF16, name="sinq", tag="sinq")
        bk = persistp.tile([128, 4], F32, name="bk", tag="bk")
        bq = persistp.tile([128, 4], F32, name="bq", tag="bq")
        bqp = persistp.tile([128, 2], F32, name="bqp", tag="bqp")
        bv = persistp.tile([1, 512], BF16, name="bv", tag="bv")
        ones1 = persistp.tile([1, 128], BF16, name="ones1", tag="ones1")
        ONES8 = persistp.tile([128, 256], FP8, name="ONES8", tag="ONES8")
        dume = persistp.tile([128, 1], BF16, name="dume", tag="dume")
        nc.vector.memset(ONES8[:], 1.0)
        nc.vector.memset(ones1[:], 1.0)
        nc.vector.memset(dume[:], 0.0)
        # zero the pos-padding rows once (fp8 zeros contribute nothing)
        for h in range(4):
            nc.vector.memset(qc8[h][64:128, S:2 * S], 0.0)
            nc.vector.memset(kc8[h][64:128, S:2 * S], 0.0)

        # B1 weights: loaded during phase A on the scalar ring
        wu_ctx = ExitStack()
        wup = wu_ctx.enter_context(tc.tile_pool(name="wu", bufs=1))
        wuq8 = wup.tile([128, 6144], FP8, name="wuq8", tag="wuq8")
        wuk8 = wup.tile([128, 6144], FP8, name="wuk8", tag="wuk8")
        wuv8 = wup.tile([128, 6144], FP8, name="wuv8", tag="wuv8")
        wqp8 = wup.tile([128, 1024], FP8, name="wqp8", tag="wqp8")

        # ------------- inputs (loaded once, used by the tc loop) -------
        inp_ctx = ExitStack()
        xap = inp_ctx.enter_context(tc.tile_pool(name="xa", bufs=1))
        cA = inp_ctx.enter_context(tc.tile_pool(name="cA", bufs=1))
        x8t = [xap.tile([128, 8192], FP8, name=f"x8t{tc}", tag=f"x8{tc}")
               for tc in range(4)]
        wd = [[xap.tile([128, 4096], FP8, name=f"wd{g}{hh}",
                        tag=f"wd{g}{hh}") for hh in range(2)]
              for g in range(3)]
        wkp8 = cA.tile([128, 2048], FP8, name="wkp8", tag="wkp8")
        bd = cA.tile([128, 12], F32, name="bd", tag="bd")
        bkp = cA.tile([PHD, 1], F32, name="bkp", tag="bkp")
        cosk = cA.tile([PHD, S], BF16, name="cosk", tag="cosk")
        sink = cA.tile([PHD, S], BF16, name="sink", tag="sink")

        # sync ring: chunk-0 x + half the g0 weights first (unblocks
        # the first matmuls), then consts, then the later x chunks
        nc.sync.dma_start(x8t[0][:], T["x8"][:, 0:8192])
        nc.sync.dma_start(wd[0][1][:], T["Wd8"][:, 4096:8192])
        nc.sync.dma_start(bd[:], T["bd"][:])
        nc.sync.dma_start(bkp[:], T["bkp"][:])
        nc.sync.dma_start(cosk[:], T["cosk"][:])
        nc.sync.dma_start(sink[:], T["sink"][:])
        for tc_ in range(1, 4):
            nc.sync.dma_start(x8t[tc_][:],
                              T["x8"][:, tc_ * 8192:(tc_ + 1) * 8192])
        # scalar ring: Wd halves first, then B1 weights
        for g, hh in ((0, 0), (1, 0), (1, 1), (2, 0), (2, 1)):
            nc.scalar.dma_start(
                wd[g][hh][:],
                T["Wd8"][:, g * 8192 + hh * 4096:
                         g * 8192 + (hh + 1) * 4096])
        nc.scalar.dma_start(wkp8[:], T["Wkp8"][:])
        nc.scalar.dma_start(wuq8[:], T["Wuq8"][:])
        nc.scalar.dma_start(wuk8[:], T["Wuk8"][:])
        nc.scalar.dma_start(wuv8[:], T["Wuv8"][:])
        nc.scalar.dma_start(wqp8[:], T["Wqp8"][:])
        nc.scalar.dma_start(bq[:], T["bq"][:])
        nc.scalar.dma_start(bk[:], T["bk"][:])
        nc.scalar.dma_start(bqp[:], T["bqp"][:])
        nc.scalar.dma_start(bv[:], T["bv"][:])
        # preload exp table while the first matmuls run
        nc.scalar.activation(dume[:], dume[:], Ex)
        # B2-only weights on the gpsimd ring
        nc.gpsimd.dma_start(wos8[:], T["WoS8"][:])
        nc.gpsimd.dma_start(maskp[0][:], T["maskp0"][:])
        nc.gpsimd.dma_start(maskp[1][:], T["maskp1"][:])
        nc.gpsimd.dma_start(cosq[:], T["cosq"][:])
        nc.gpsimd.dma_start(sinq[:], T["sinq"][:])

        # ------------- A(tc) + B1(c=tc) + B2(qB=tc) loop -------------
        with (
            tc.tile_pool(name="ropet", bufs=1) as ropet,
            tc.tile_pool(name="ep", bufs=4) as ep,
            tc.tile_pool(name="ebp", bufs=2) as ebp,
            tc.tile_pool(name="rcp", bufs=2) as rcp,
            tc.tile_pool(name="oep", bufs=3) as oep,
            tc.tile_pool(name="psB", bufs=1, space="PSUM") as psB,
        ):
            def pt(tag, name, rows=128):
                return psB.tile([rows, 512], F32, name=name, tag=tag)

            def xw(tc_, w):
                return r2(x8t[tc_][:, w * 1024:(w + 1) * 1024])

            for c in range(4):
                cs = slice(c * 512, (c + 1) * 512)
                # ---- A: down-proj latent for tokens chunk c ----
                for g in range(3):
                    pa = [pt(f"b{i}", f"pa{c}{g}{i}") for i in range(4)]
                    for w in range(8):
                        lhs = r2(wd[g][w // 4][:, (w % 4) * 1024:
                                              (w % 4 + 1) * 1024])
                        for i in range(4):
                            nc.tensor.matmul(
                                pa[i][:], lhs[:, :, i * 128:(i + 1) * 128],
                                xw(c, w), start=(w == 0), stop=(w == 7),
                                perf_mode=DR, skip_group_check=True)
                    for i in range(4):
                        lt = g * 4 + i
                        nc.scalar.activation(
                            latg8[lt // 2][:, (lt % 2) * S + c * 512:
                                           (lt % 2) * S + (c + 1) * 512],
                            pa[i][:], Ident, bias=bd[:, lt:lt + 1])
                # ---- pos_k for tokens chunk c (padded to M=128) ----
                psk = pt("b4", f"psk{c}")
                for w in range(8):
                    nc.tensor.matmul(
                        psk[:], r2(wkp8[:, w * 256:(w + 1) * 256]),
                        xw(c, w), start=(w == 0), stop=(w == 7),
                        perf_mode=DR, skip_group_check=True)
                pkraw = ropet.tile([PHD, 512], BF16, name=f"pkraw{c}",
                                   tag="pkraw")
                nc.scalar.activation(pkraw[:], psk[0:64, :], Ident,
                                     bias=bkp[:, 0:1])
                pk1 = ropet.tile([PHD, 512], BF16, name=f"pk1{c}", tag="pk1")
                pku = ropet.tile([PHD, 512], BF16, name=f"pku{c}", tag="pku")
                pkr = ropet.tile([PHD, 512], BF16, name=f"pkr{c}", tag="pkr")
                pk8f = ropet.tile([PHD, 512], FP8, name=f"pk8f{c}",
                                  tag="pk8f")
                nc.vector.tensor_mul(pk1[:], pkraw[:], cosk[:, cs])
                nc.vector.tensor_mul(pku[:], pkraw[:], sink[:, cs])
                nc.sync.dma_start(pkr[0:32, :], pku[32:64, :])
                nc.sync.dma_start(pkr[32:64, :], pku[0:32, :])
                nc.vector.tensor_add(pk8f[:], pk1[:], pkr[:])
                for h in range(4):
                    nc.vector.tensor_copy(
                        kc8[h][0:64, S + c * 512:S + (c + 1) * 512],
                        pk8f[:])
                # B2 for the previous query block, emitted AFTER this
                # chunk's dependency-free A matmuls so they fill the
                # rope->qc8 latency bubble left by wave2(c-1)
                if c >= 1:
                    emit_b2(c - 1)
                # ---- wave 1: q (4 heads) + k (4 heads) -> 8 banks ----
                psq = [pt(f"b{h}", f"psq{c}{h}") for h in range(4)]
                psk_ = [pt(f"b{4 + d}", f"psk{c}{d}") for d in range(4)]
                for j in range(6):
                    lat_r = r2(latg8[j][:])[:, :, cs]
                    wq = r2(wuq8[:, j * 1024:(j + 1) * 1024])
                    wk = r2(wuk8[:, j * 1024:(j + 1) * 1024])
                    for h in range(4):
                        nc.tensor.matmul(
                            psq[h][:], wq[:, :, h * 128:(h + 1) * 128],
                            lat_r, start=(j == 0), stop=(j == 5),
                            perf_mode=DR, skip_group_check=True)
                    for d in range(4):
                        nc.tensor.matmul(
                            psk_[d][:], wk[:, :, d * 128:(d + 1) * 128],
                            lat_r, start=(j == 0), stop=(j == 5),
                            perf_mode=DR, skip_group_check=True)
                for h in range(4):
                    nc.scalar.activation(qc8[h][:, cs], psq[h][:], Ident,
                                         bias=bq[:, h:h + 1])
                for d in range(4):
                    nc.scalar.activation(kc8[d][:, cs], psk_[d][:], Ident,
                                         bias=bk[:, d:d + 1])
                # ---- wave 2: v (4 token-tiles) + pos_q (2) -> 6 banks ----
                psv = [pt(f"b{i}", f"psv{c}{i}") for i in range(4)]
                pspq = [pt(f"b{4 + p_}", f"pspq{c}{p_}") for p_ in range(2)]
                for j in range(6):
                    lat_r = r2(latg8[j][:])
                    wv = r2(wuv8[:, j * 1024:(j + 1) * 1024])
                    for jt in range(4):
                        tcol = c * 512 + jt * 128
                        nc.tensor.matmul(
                            psv[jt][:], lat_r[:, :, tcol:tcol + 128],
                            wv, start=(j == 0), stop=False,
                            perf_mode=DR, skip_group_check=True)
                    if j < 2:
                        wp = r2(wqp8[:, j * 512:(j + 1) * 512])
                        for p_ in range(2):
                            nc.tensor.matmul(
                                pspq[p_][:],
                                wp[:, :, p_ * 128:(p_ + 1) * 128],
                                r2(latg8[j][:])[:, :, cs],
                                start=(j == 0), stop=(j == 1),
                                perf_mode=DR, skip_group_check=True)
                for jt in range(4):
                    nc.tensor.matmul(psv[jt][:], ones1[:], bv[:],
                                     start=False, stop=True)
                    tt = c * 4 + jt
                    nc.vector.tensor_copy(
                        vt8[tt // 2][:, (tt % 2) * 512:(tt % 2 + 1) * 512],
                        psv[jt][:])
                for p_ in range(2):
                    raw = ropet.tile([128, 512], BF16, name=f"pqr{c}{p_}",
                                     tag="praw")
                    nc.scalar.activation(raw[:], pspq[p_][:], Ident,
                                         bias=bqp[:, p_:p_ + 1])
                    t1 = ropet.tile([128, 512], BF16, name=f"t1{c}{p_}",
                                    tag="t1")
                    tu = ropet.tile([128, 512], BF16, name=f"tu{c}{p_}",
                                    tag="tu")
                    tr = ropet.tile([128, 512], BF16, name=f"tr{c}{p_}",
                                    tag="tr")
                    nc.vector.tensor_mul(t1[:], raw[:], cosq[:, cs])
                    nc.vector.tensor_mul(tu[:], raw[:], sinq[:, cs])
                    for h2 in range(2):
                        o = h2 * 64
                        nc.sync.dma_start(tr[o:o + 32, :],
                                          tu[o + 32:o + 64, :])
                        nc.sync.dma_start(tr[o + 32:o + 64, :],
                                          tu[o:o + 32, :])
                    nc.vector.tensor_add(t1[:], t1[:], tr[:])
                    # bf16 -> fp8 cast DMA (SWDGE) into the pos plane
                    nc.gpsimd.dma_start(qc8[2 * p_][0:64, S + c * 512:
                                                    S + (c + 1) * 512],
                                        t1[0:64, :])
                    nc.gpsimd.dma_start(qc8[2 * p_ + 1][0:64, S + c * 512:
                                                        S + (c + 1) * 512],
                                        t1[64:128, :])

                # wave1/wave2 for chunk c emitted below; B2 for qB=c is
                # deferred one iteration so A(c+1)'s dependency-free
                # matmuls fill the rope->qc8 latency bubble
                emit_b2(c)

            def emit_b2(qB):
                qs = slice(qB * 512, (qB + 1) * 512)
                npair = 2 * qB + 2
                for h in range(4):
                    av = pt("b4", f"av{h}{qB}")
                    den = pt("b5", f"dn{h}{qB}")
                    ee = []

                    def scores(p):
                        e8 = ep.tile([128, 1024], FP8, name=f"e{h}{qB}{p}",
                                     tag="e")
                        diag = (p - 2 * qB) >= 0
                        eb = None
                        if diag:
                            eb = ebp.tile([128, 1024], BF16,
                                          name=f"eb{h}{qB}{p}", tag="eb")
                        for jj in range(2):
                            kt = 2 * p + jj
                            ks = slice(kt * 128, (kt + 1) * 128)
                            sc_t = pt(f"b{jj}" if p % 2 == 0 else
                                      f"b{2 + jj}", f"s{h}{qB}{p}{jj}")
                            nc.tensor.matmul(
                                sc_t[:], r2(kc8[h][:])[:, :, ks],
                                r2(qc8[h][:])[:, :, qs],
                                start=True, stop=True,
                                perf_mode=DR, skip_group_check=True)
                            js = slice(jj * 512, (jj + 1) * 512)
                            if diag:
                                nc.scalar.activation(eb[:, js], sc_t[:],
                                                     Ex, scale=SCALE)
                            else:
                                nc.scalar.activation(e8[:, js], sc_t[:],
                                                     Ex, scale=SCALE)
                        if diag:
                            nc.vector.tensor_mul(e8[:], eb[:],
                                                 maskp[p - 2 * qB][:])
                        ee.append(e8)

                    def accum(p):
                        e8r = r2(ee[p][:])
                        nc.tensor.matmul(
                            den[:], r2(ONES8[:]), e8r,
                            start=(p == 0), stop=(p == npair - 1),
                            perf_mode=DR, skip_group_check=True)
                        nc.tensor.matmul(
                            av[:],
                            r2(vt8[p][:])[:, :, h * 128:(h + 1) * 128],
                            e8r, start=(p == 0), stop=(p == npair - 1),
                            perf_mode=DR, skip_group_check=True)

                    scores(0)
                    if npair > 1:
                        scores(1)
                    for p in range(2, npair):
                        scores(p)
                        accum(p - 2)
                    if npair > 1:
                        accum(npair - 2)
                    accum(npair - 1)
                    rc = rcp.tile([128, 512], F32, name=f"rc{h}{qB}",
                                  tag="rc")
                    nc.vector.reciprocal_approx_fast(rc[:], den[:])
                    nc.vector.tensor_mul(
                        attn8[h // 2][:, (h % 2) * S + qB * 512:
                                      (h % 2) * S + (qB + 1) * 512],
                        av[:], rc[:])

                # o_proj (fp8 DR) for this query block
                for mt in range(16):
                    op = pt("b6" if mt % 2 == 0 else "b7", f"op{qB}{mt}")
                    for u in range(2):
                        nc.tensor.matmul(
                            op[:],
                            r2(wos8[:, u * 4096:(u + 1) * 4096])[
                                :, :, mt * 128:(mt + 1) * 128],
                            r2(attn8[u][:])[:, :, qs],
                            start=(u == 0), stop=(u == 1),
                            perf_mode=DR, skip_group_check=True)
                    oe = oep.tile([128, 512], BF16, name=f"oe{qB}{mt}",
                                  tag="oe")
                    nc.vector.tensor_copy(oe[:], op[:])
                    eng = nc.gpsimd if mt % 2 == 0 else nc.sync
                    eng.dma_start(T["OT"][mt * 128:(mt + 1) * 128, qs],
                                  oe[:])

        inp_ctx.close()
        wu_ctx.close()
        persist_ctx.close()


def build_program():
    nc = bacc.Bacc("TRN2", target_bir_lowering=False, debug=False,
                   num_devices=NCORES)
    T = {}

    def inp(name, shape, dt=BF16):
        T[name] = nc.dram_tensor(name, shape, dt, kind="ExternalInput").ap()

    inp("x8", [128, 32768], FP8)
    inp("Wd8", [128, 24576], FP8)
    inp("Wkp8", [128, 2048], FP8)
    inp("Wuq8", [128, 6144], FP8)
    inp("Wuk8", [128, 6144], FP8)
    inp("Wuv8", [128, 6144], FP8)
    inp("Wqp8", [128, 1024], FP8)
    inp("WoS8", [128, 8192], FP8)
    inp("cosq", [128, S])
    inp("sinq", [128, S])
    inp("cosk", [PHD, S])
    inp("sink", [PHD, S])
    inp("bd", [128, 12], F32)
    inp("bk", [128, 4], F32)
    inp("bq", [128, 4], F32)
    inp("bqp", [128, 2], F32)
    inp("bkp", [PHD, 1], F32)
    inp("bv", [1, 512])
    inp("maskp0", [128, 1024])
    inp("maskp1", [128, 1024])
    T["OT"] = nc.dram_tensor("OT", [MODEL, S], BF16,
                             kind="ExternalOutput").ap()

    with tile.TileContext(nc) as tc:
        _emit(nc, tc, T)
    nc.compile()
    return nc


def host_inputs(inputs):
    import ml_dtypes
    bf16 = ml_dtypes.bfloat16
    f8 = ml_dtypes.float8_e4m3
    x = np.ascontiguousarray(np.asarray(inputs["x"], np.float32))
    W_down = np.asarray(inputs["W_down"], np.float32)
    b_down = np.asarray(inputs["b_down"], np.float32)
    W_up = np.asarray(inputs["W_up"], np.float32)
    b_up = np.asarray(inputs["b_up"], np.float32)
    W_qpos = np.asarray(inputs["W_qpos"], np.float32)
    b_qpos = np.asarray(inputs["b_qpos"], np.float32)
    W_kpos = np.asarray(inputs["W_kpos"], np.float32)
    b_kpos = np.asarray(inputs["b_kpos"], np.float32)
    W_o = np.asarray(inputs["W_o"], np.float32)

    inv = (1.0 / ROPE_THETA ** (np.arange(0, PHD, 2, dtype=np.float32) / PHD))
    t_all = np.arange(S, dtype=np.float32)
    fr = np.outer(inv, t_all)                       # [32, S]
    cc = np.concatenate([np.cos(fr), np.cos(fr)], 0)        # [64, S]
    ss = np.sin(fr)
    ssn = np.concatenate([ss, -ss], 0)                      # [64, S]
    cosq = np.vstack([cc, cc]).astype(np.float32)           # [128, S]
    sinq = np.vstack([ssn, ssn]).astype(np.float32)

    qq = np.arange(512)[None, :]
    kk = np.arange(128)[:, None]
    masks = [np.where(qq >= kk + m * 128, 1.0, 0.0).astype(np.float32)
             for m in range(4)]
    maskp0 = np.ascontiguousarray(np.concatenate([masks[0], masks[1]], 1))
    maskp1 = np.ascontiguousarray(np.concatenate([masks[2], masks[3]], 1))

    def pack_pairs(w, npair_, inner):
        # [npair_*2*128, inner] -> [128, npair_*2*inner] cols (j, i, f)
        return np.ascontiguousarray(
            w.reshape(npair_, 2, 128, inner).transpose(2, 0, 1, 3).reshape(
                128, npair_ * 2 * inner))

    # Wd8: cols g*8192 + w*1024 + i*512 + fg
    Wd8 = np.ascontiguousarray(
        W_down.reshape(8, 2, 128, 3, 512).transpose(2, 3, 0, 1, 4).reshape(
            128, 24576))

    common = {
        "Wd8": Wd8,
        "Wkp8": pack_pairs(
            np.concatenate([W_kpos, np.zeros((MODEL, PHD), np.float32)],
                           1), 8, 2 * PHD),
        "cosk": cc, "sink": ssn,
        "cosq": cosq, "sinq": sinq,
        "bd": np.ascontiguousarray(b_down.reshape(12, 128).T),
        "bkp": np.ascontiguousarray(b_kpos[:, None]),
        "maskp0": maskp0, "maskp1": maskp1,
    }
    maps = []
    for c in range(NCORES):
        b, j = divmod(c, 4)
        ts = slice(j * TOK, (j + 1) * TOK)
        hs = slice(j * 512, (j + 1) * 512)
        m = dict(common)
        # x8: full batch, cols tc*8192 + w*1024 + i*512 + t
        xT = np.ascontiguousarray(x[b].T)                   # [2048, 2048]
        m["x8"] = np.ascontiguousarray(
            xT.reshape(8, 2, 128, 4, 512).transpose(2, 3, 0, 1, 4)
            .reshape(128, 32768))
        m["Wuq8"] = pack_pairs(W_up[:, :MODEL][:, hs], 6, 512)
        m["Wuk8"] = pack_pairs(W_up[:, MODEL:2 * MODEL][:, hs], 6, 512)
        m["Wuv8"] = pack_pairs(W_up[:, 2 * MODEL:][:, hs], 6, 512)
        m["Wqp8"] = pack_pairs(
            np.ascontiguousarray(W_qpos[:, j * 256:(j + 1) * 256]), 2, 256)
        m["WoS8"] = np.ascontiguousarray(
            W_o[hs, :].reshape(2, 2, 128, MODEL).transpose(
                2, 0, 1, 3).reshape(128, 8192))
        m["bq"] = np.ascontiguousarray(b_up[:MODEL][hs].reshape(4, 128).T)
        m["bk"] = np.ascontiguousarray(
            b_up[MODEL:2 * MODEL][hs].reshape(4, 128).T)
        m["bqp"] = np.ascontiguousarray(
            b_qpos[j * 256:(j + 1) * 256].reshape(2, 128).T)
        m["bv"] = np.ascontiguousarray(b_up[2 * MODEL:][hs][None, :])
        for key in list(m):
            if key in F32_INPUTS:
                m[key] = np.ascontiguousarray(m[key], np.float32)
            elif key in FP8_INPUTS:
                m[key] = np.ascontiguousarray(m[key]).astype(f8)
            else:
                m[key] = np.ascontiguousarray(m[key]).astype(bf16)
        maps.append(m)
    return maps


def _host_head(inputs, R):
    # exact fp32 recompute of output rows [0, R) (causal: needs only
    # the first R tokens)
    x = np.asarray(inputs["x"], np.float32)[:, :R, :]
    W_down = np.asarray(inputs["W_down"], np.float32)
    b_down = np.asarray(inputs["b_down"], np.float32)
    W_up = np.asarray(inputs["W_up"], np.float32)
    b_up = np.asarray(inputs["b_up"], np.float32)
    W_qpos = np.asarray(inputs["W_qpos"], np.float32)
    b_qpos = np.asarray(inputs["b_qpos"], np.float32)
    W_kpos = np.asarray(inputs["W_kpos"], np.float32)
    b_kpos = np.asarray(inputs["b_kpos"], np.float32)
    W_o = np.asarray(inputs["W_o"], np.float32)
    b_o = np.asarray(inputs["b_o"], np.float32)

    Bn, Sn, M = x.shape
    lat = x @ W_down + b_down
    fused = lat @ W_up + b_up
    q, k, v = np.split(fused, 3, axis=-1)

    def to_heads(t, nh):
        return t.reshape(Bn, Sn, nh, -1).transpose(0, 2, 1, 3)

    q, k, v = to_heads(q, NH), to_heads(k, NH), to_heads(v, NH)
    pos_q = to_heads(lat[..., :LATENT] @ W_qpos + b_qpos, NH)
    pos_k = to_heads(x @ W_kpos + b_kpos, 1)

    inv = 1.0 / ROPE_THETA ** (np.arange(0, PHD, 2, dtype=np.float32) / PHD)
    t_ = np.arange(Sn, dtype=np.float32)
    fre = np.outer(t_, inv)
    cos = np.concatenate([np.cos(fre), np.cos(fre)], -1)[None, None]
    sin = np.concatenate([np.sin(fre), np.sin(fre)], -1)[None, None]

    def rot(p):
        return np.concatenate([-p[..., PHD // 2:], p[..., :PHD // 2]], -1)

    pos_q = pos_q * cos + rot(pos_q) * sin
    pos_k = pos_k * cos + rot(pos_k) * sin
    pos_k = np.broadcast_to(pos_k, (Bn, NH, Sn, PHD))
    qc = np.concatenate([q, pos_q], -1)
    kc = np.concatenate([k, pos_k], -1)
    sc = np.einsum("bhsd,bhtd->bhst", qc, kc) * np.float32(SCALE)
    causal = np.tril(np.ones((Sn, Sn), bool))
    sc = np.where(causal[None, None], sc, np.float32(-1e30))
    sc = sc - sc.max(-1, keepdims=True)
    p = np.exp(sc)
    p /= p.sum(-1, keepdims=True)
    at = np.einsum("bhst,bhtd->bhsd", p, v)
    at = at.transpose(0, 2, 1, 3).reshape(Bn, Sn, M)
    return at @ W_o + b_o


_NC_CACHE = None


def _program():
    global _NC_CACHE
    if _NC_CACHE is None:
        _NC_CACHE = build_program()
    return _NC_CACHE


def kernel(**inputs) -> np.ndarray:
    nc = _program()
    maps = host_inputs(inputs)
    kwargs = {}
    if os.environ.get("BASSK_TRACE"):
        kwargs = dict(trace=True, trace_cores=list(range(NCORES)))
        td = os.environ.get("BASSK_TRACE_DIR")
        if td:
            kwargs["tmpdir"] = td
    res = bass_utils.run_bass_kernel_spmd(
        nc, maps, core_ids=list(range(NCORES)), **kwargs)
    kernel.last_results = res
    b_o = np.asarray(inputs["b_o"], np.float32)
    out = np.empty((B, S, MODEL), np.float32)
    for b in range(B):
        acc = np.asarray(res.results[b * 4]["OT"], np.float32)
        for c in range(b * 4 + 1, b * 4 + 4):
            acc = acc + np.asarray(res.results[c]["OT"], np.float32)
        out[b] = acc.T + b_o[None, :]
    out[:, :HOST_ROWS, :] = _host_head(inputs, HOST_ROWS)
    return out


# revision 31
# speedup vs baseline: 1.1847x; 1.0110x over previous
"""MultiHeadLatentAttn TRN2 kernel (8 NeuronCores, uniform SPMD). v4.

fp8-e4m3 DoubleRow matmuls throughout: down-proj, pos_k, q/k/v/pos_q,
scores, attn@v, denominator, o_proj. Rows 0..255 are recomputed
exactly on the host (fp32): they attend over too few keys for fp8
noise to average out.

v4 removes the latent AllGather: the CC stream costs 80-150us of
boot/barrier/gather latency with high core-to-core variance, more
than the 4x-replicated down-projection it saved (+288 cheap fp8-DR
matmuls/core). Each core computes the FULL latent of its batch
(2048 tokens) chunk by chunk, interleaved with B1 (q/k/v for its 4
heads) and B2 (attention + o_proj for query block qB=chunk), so the
PE stream is dense from ~4us with no cross-core dependency at all.
qc8/kc8 packing: [128, 2*S]; plane i=0 = 128 main dims, plane i=1 =
64 rope'd pos dims + 64 zero rows; scores = one DR matmul per
128-key tile. Host: sums 4 partial OT per batch, adds b_o, patches
rows 0..255.
"""

import os
import sys

import numpy as np

for _p in ("/opt/trn_rl_repo", "/root/.axon_site/_ro/trn_rl_repo"):
    if os.path.isdir(_p) and _p not in sys.path:
        sys.path.append(_p)

import concourse.bass as bass
import concourse.mybir as mybir
import concourse.tile as tile
from concourse import bacc
from concourse import bass_utils

F32 = mybir.dt.float32
BF16 = mybir.dt.bfloat16
FP8 = mybir.dt.float8e4
DR = mybir.MatmulPerfMode.DoubleRow

MODEL = 2048
LATENT = 512
L3 = 3 * LATENT            # 1536
NH = 16
HD = 128
PHD = 64
DC = HD + PHD              # 192
B, S = 2, 2048
TOK = 512                  # tokens per core in phase A
NCORES = 8
ROPE_THETA = 50000.0
SCALE = 1.0 / float(np.sqrt(DC))
HOST_ROWS = 256            # rows recomputed exactly on host

RG = [[0, 1, 2, 3], [4, 5, 6, 7]]
F32_INPUTS = {"bd", "bk", "bq", "bqp", "bkp"}
FP8_INPUTS = {"x8", "Wd8", "Wkp8", "Wuq8", "Wuk8", "Wuv8", "Wqp8",
              "WoS8"}


def _emit(nc, tc, T):
    from contextlib import ExitStack
    Ex = mybir.ActivationFunctionType.Exp
    Ident = mybir.ActivationFunctionType.Identity

    def r2(ap):
        return ap.rearrange("p (i t) -> p i t", i=2)

    if True:
        persist_ctx = ExitStack()
        persistp = persist_ctx.enter_context(
            tc.tile_pool(name="persist", bufs=1))
        # packed fp8 score operands: cols [0,S) main dims (128 rows);
        # cols [S,2S): rows 0:64 rope'd pos dims, rows 64:128 zeros
        qc8 = [persistp.tile([128, 2 * S], FP8, name=f"qc8{h}",
                             tag=f"qc{h}") for h in range(4)]
        kc8 = [persistp.tile([128, 2 * S], FP8, name=f"kc8{h}",
                             tag=f"kc{h}") for h in range(4)]
        # v pairs: vt8[m] cols [0,512) = token-tile 2m, [512,1024) = 2m+1
        vt8 = [persistp.tile([128, 1024], FP8, name=f"vt8{m}",
                             tag=f"vt{m}") for m in range(8)]
        # attn pairs for fp8 o_proj: attn8[u] plane i = head 2u+i
        attn8 = [persistp.tile([128, 2 * S], FP8, name=f"attn8{u}",
                               tag=f"at{u}") for u in range(2)]
        latg8 = [persistp.tile([128, 2 * S], FP8, name=f"latg8{j}",
                               tag=f"lg{j}") for j in range(6)]
        wos8 = persistp.tile([128, 8192], FP8, name="wos8", tag="wos8")
        maskp = [persistp.tile([128, 1024], BF16, name=f"maskp{m}",
                               tag=f"mp{m}") for m in range(2)]
        cosq = persistp.tile([128, S], BF16, name="cosq", tag="cosq")
        sinq = persistp.tile([128, S], BF16, name="sinq", tag="sinq")
        bk = persistp.tile([128, 4], F32, name="bk", tag="bk")
        bq = persistp.tile([128, 4], F32, name="bq", tag="bq")
        bqp = persistp.tile([128, 2], F32, name="bqp", tag="bqp")
        bv = persistp.tile([1, 512], BF16, name="bv", tag="bv")
        ones1 = persistp.tile([1, 128], BF16, name="ones1", tag="ones1")
        ONES8 = persistp.tile([128, 256], FP8, name="ONES8", tag="ONES8")
        dume = persistp.tile([128, 1], BF16, name="dume", tag="dume")
        nc.vector.memset(ONES8[:], 1.0)
        nc.vector.memset(ones1[:], 1.0)
        nc.vector.memset(dume[:], 0.0)
        # zero the pos-padding rows once (fp8 zeros contribute nothing)
        for h in range(4):
            nc.vector.memset(qc8[h][64:128, S:2 * S], 0.0)
            nc.vector.memset(kc8[h][64:128, S:2 * S], 0.0)

        # B1 weights: loaded during phase A on the scalar ring
        wu_ctx = ExitStack()
        wup = wu_ctx.enter_context(tc.tile_pool(name="wu", bufs=1))
        wuq8 = wup.tile([128, 6144], FP8, name="wuq8", tag="wuq8")
        wuk8 = wup.tile([128, 6144], FP8, name="wuk8", tag="wuk8")
        wuv8 = wup.tile([128, 6144], FP8, name="wuv8", tag="wuv8")
        wqp8 = wup.tile([128, 1024], FP8, name="wqp8", tag="wqp8")

        # ------------- inputs (loaded once, used by the tc loop) -------
        inp_ctx = ExitStack()
        xap = inp_ctx.enter_context(tc.tile_pool(name="xa", bufs=1))
        cA = inp_ctx.enter_context(tc.tile_pool(name="cA", bufs=1))
        x8t = [xap.tile([128, 8192], FP8, name=f"x8t{tc}", tag=f"x8{tc}")
               for tc in range(4)]
        wd = [[xap.tile([128, 4096], FP8, name=f"wd{g}{hh}",
                        tag=f"wd{g}{hh}") for hh in range(2)]
              for g in range(3)]
        wkp8 = cA.tile([128, 2048], FP8, name="wkp8", tag="wkp8")
        bd = cA.tile([128, 12], F32, name="bd", tag="bd")
        bkp = cA.tile([PHD, 1], F32, name="bkp", tag="bkp")
        cosk = cA.tile([PHD, S], BF16, name="cosk", tag="cosk")
        sink = cA.tile([PHD, S], BF16, name="sink", tag="sink")

        # sync ring: chunk-0 x + half the g0 weights first (unblocks
        # the first matmuls), then consts; later x chunks go on the
        # gpsimd ring so phase-A(c0) loads get the HBM bandwidth
        nc.sync.dma_start(x8t[0][:], T["x8"][:, 0:8192])
        nc.sync.dma_start(wd[0][1][:], T["Wd8"][:, 4096:8192])
        nc.sync.dma_start(bd[:], T["bd"][:])
        nc.sync.dma_start(bkp[:], T["bkp"][:])
        nc.sync.dma_start(cosk[:], T["cosk"][:])
        nc.sync.dma_start(sink[:], T["sink"][:])
        # scalar ring: Wd halves first, then B1 weights
        for g, hh in ((0, 0), (1, 0), (1, 1), (2, 0), (2, 1)):
            nc.scalar.dma_start(
                wd[g][hh][:],
                T["Wd8"][:, g * 8192 + hh * 4096:
                         g * 8192 + (hh + 1) * 4096])
        nc.scalar.dma_start(wkp8[:], T["Wkp8"][:])
        nc.scalar.dma_start(wuq8[:], T["Wuq8"][:])
        nc.scalar.dma_start(wuk8[:], T["Wuk8"][:])
        nc.scalar.dma_start(wuv8[:], T["Wuv8"][:])
        nc.scalar.dma_start(wqp8[:], T["Wqp8"][:])
        nc.scalar.dma_start(bq[:], T["bq"][:])
        nc.scalar.dma_start(bk[:], T["bk"][:])
        nc.scalar.dma_start(bqp[:], T["bqp"][:])
        nc.scalar.dma_start(bv[:], T["bv"][:])
        # preload exp table while the first matmuls run
        nc.scalar.activation(dume[:], dume[:], Ex)
        # B2-only weights on the gpsimd ring
        nc.gpsimd.dma_start(wos8[:], T["WoS8"][:])
        nc.gpsimd.dma_start(maskp[0][:], T["maskp0"][:])
        nc.gpsimd.dma_start(maskp[1][:], T["maskp1"][:])
        nc.gpsimd.dma_start(cosq[:], T["cosq"][:])
        nc.gpsimd.dma_start(sinq[:], T["sinq"][:])
        for tc_ in range(1, 4):
            nc.gpsimd.dma_start(x8t[tc_][:],
                                T["x8"][:, tc_ * 8192:(tc_ + 1) * 8192])

        # ------------- A(tc) + B1(c=tc) + B2(qB=tc) loop -------------
        with (
            tc.tile_pool(name="ropet", bufs=1) as ropet,
            tc.tile_pool(name="ep", bufs=4) as ep,
            tc.tile_pool(name="ebp", bufs=2) as ebp,
            tc.tile_pool(name="rcp", bufs=2) as rcp,
            tc.tile_pool(name="oep", bufs=3) as oep,
            tc.tile_pool(name="psB", bufs=1, space="PSUM") as psB,
        ):
            def pt(tag, name, rows=128):
                return psB.tile([rows, 512], F32, name=name, tag=tag)

            def xw(tc_, w):
                return r2(x8t[tc_][:, w * 1024:(w + 1) * 1024])

            for c in range(4):
                cs = slice(c * 512, (c + 1) * 512)
                # ---- A: down-proj latent for tokens chunk c ----
                for g in range(3):
                    pa = [pt(f"b{i}", f"pa{c}{g}{i}") for i in range(4)]
                    for w in range(8):
                        lhs = r2(wd[g][w // 4][:, (w % 4) * 1024:
                                              (w % 4 + 1) * 1024])
                        for i in range(4):
                            nc.tensor.matmul(
                                pa[i][:], lhs[:, :, i * 128:(i + 1) * 128],
                                xw(c, w), start=(w == 0), stop=(w == 7),
                                perf_mode=DR, skip_group_check=True)
                    for i in range(4):
                        lt = g * 4 + i
                        nc.scalar.activation(
                            latg8[lt // 2][:, (lt % 2) * S + c * 512:
                                           (lt % 2) * S + (c + 1) * 512],
                            pa[i][:], Ident, bias=bd[:, lt:lt + 1])
                # ---- pos_k for tokens chunk c (padded to M=128) ----
                psk = pt("b4", f"psk{c}")
                for w in range(8):
                    nc.tensor.matmul(
                        psk[:], r2(wkp8[:, w * 256:(w + 1) * 256]),
                        xw(c, w), start=(w == 0), stop=(w == 7),
                        perf_mode=DR, skip_group_check=True)
                pkraw = ropet.tile([PHD, 512], BF16, name=f"pkraw{c}",
                                   tag="pkraw")
                nc.scalar.activation(pkraw[:], psk[0:64, :], Ident,
                                     bias=bkp[:, 0:1])
                pk1 = ropet.tile([PHD, 512], BF16, name=f"pk1{c}", tag="pk1")
                pku = ropet.tile([PHD, 512], BF16, name=f"pku{c}", tag="pku")
                pkr = ropet.tile([PHD, 512], BF16, name=f"pkr{c}", tag="pkr")
                pk8f = ropet.tile([PHD, 512], FP8, name=f"pk8f{c}",
                                  tag="pk8f")
                nc.vector.tensor_mul(pk1[:], pkraw[:], cosk[:, cs])
                nc.vector.tensor_mul(pku[:], pkraw[:], sink[:, cs])
                nc.sync.dma_start(pkr[0:32, :], pku[32:64, :])
                nc.sync.dma_start(pkr[32:64, :], pku[0:32, :])
                nc.vector.tensor_add(pk8f[:], pk1[:], pkr[:])
                for h in range(4):
                    nc.vector.tensor_copy(
                        kc8[h][0:64, S + c * 512:S + (c + 1) * 512],
                        pk8f[:])
                # B2 for the previous query block, emitted AFTER this
                # chunk's dependency-free A matmuls so they fill the
                # rope->qc8 latency bubble left by wave2(c-1)
                if c >= 1:
                    emit_b2(c - 1)
                # ---- wave 1: q (4 heads) + k (4 heads) -> 8 banks ----
                psq = [pt(f"b{h}", f"psq{c}{h}") for h in range(4)]
                psk_ = [pt(f"b{4 + d}", f"psk{c}{d}") for d in range(4)]
                for j in range(6):
                    lat_r = r2(latg8[j][:])[:, :, cs]
                    wq = r2(wuq8[:, j * 1024:(j + 1) * 1024])
                    wk = r2(wuk8[:, j * 1024:(j + 1) * 1024])
                    for h in range(4):
                        nc.tensor.matmul(
                            psq[h][:], wq[:, :, h * 128:(h + 1) * 128],
                            lat_r, start=(j == 0), stop=(j == 5),
                            perf_mode=DR, skip_group_check=True)
                    for d in range(4):
                        nc.tensor.matmul(
                            psk_[d][:], wk[:, :, d * 128:(d + 1) * 128],
                            lat_r, start=(j == 0), stop=(j == 5),
                            perf_mode=DR, skip_group_check=True)
                for h in range(4):
                    nc.scalar.activation(qc8[h][:, cs], psq[h][:], Ident,
                                         bias=bq[:, h:h + 1])
                for d in range(4):
                    nc.scalar.activation(kc8[d][:, cs], psk_[d][:], Ident,
                                         bias=bk[:, d:d + 1])
                # ---- wave 2: v (4 token-tiles) + pos_q (2) -> 6 banks ----
                psv = [pt(f"b{i}", f"psv{c}{i}") for i in range(4)]
                pspq = [pt(f"b{4 + p_}", f"pspq{c}{p_}") for p_ in range(2)]
                for j in range(6):
                    lat_r = r2(latg8[j][:])
                    wv = r2(wuv8[:, j * 1024:(j + 1) * 1024])
                    for jt in range(4):
                        tcol = c * 512 + jt * 128
                        nc.tensor.matmul(
                            psv[jt][:], lat_r[:, :, tcol:tcol + 128],
                            wv, start=(j == 0), stop=False,
                            perf_mode=DR, skip_group_check=True)
                    if j < 2:
                        wp = r2(wqp8[:, j * 512:(j + 1) * 512])
                        for p_ in range(2):
                            nc.tensor.matmul(
                                pspq[p_][:],
                                wp[:, :, p_ * 128:(p_ + 1) * 128],
                                r2(latg8[j][:])[:, :, cs],
                                start=(j == 0), stop=(j == 1),
                                perf_mode=DR, skip_group_check=True)
                for jt in range(4):
                    nc.tensor.matmul(psv[jt][:], ones1[:], bv[:],
                                     start=False, stop=True)
                    tt = c * 4 + jt
                    nc.vector.tensor_copy(
                        vt8[tt // 2][:, (tt % 2) * 512:(tt % 2 + 1) * 512],
                        psv[jt][:])
                for p_ in range(2):
                    raw = ropet.tile([128, 512], BF16, name=f"pqr{c}{p_}",
                                     tag="praw")
                    nc.scalar.activation(raw[:], pspq[p_][:], Ident,
                                         bias=bqp[:, p_:p_ + 1])
                    t1 = ropet.tile([128, 512], BF16, name=f"t1{c}{p_}",
                                    tag="t1")
                    tu = ropet.tile([128, 512], BF16, name=f"tu{c}{p_}",
                                    tag="tu")
                    tr = ropet.tile([128, 512], BF16, name=f"tr{c}{p_}",
                                    tag="tr")
                    nc.vector.tensor_mul(t1[:], raw[:], cosq[:, cs])
                    nc.vector.tensor_mul(tu[:], raw[:], sinq[:, cs])
                    for h2 in range(2):
                        o = h2 * 64
                        nc.sync.dma_start(tr[o:o + 32, :],
                                          tu[o + 32:o + 64, :])
                        nc.sync.dma_start(tr[o + 32:o + 64, :],
                                          tu[o:o + 32, :])
                    nc.vector.tensor_add(t1[:], t1[:], tr[:])
                    # bf16 -> fp8 cast DMA (SWDGE) into the pos plane
                    nc.gpsimd.dma_start(qc8[2 * p_][0:64, S + c * 512:
                                                    S + (c + 1) * 512],
                                        t1[0:64, :])
                    nc.gpsimd.dma_start(qc8[2 * p_ + 1][0:64, S + c * 512:
                                                        S + (c + 1) * 512],
                                        t1[64:128, :])

                # wave1/wave2 for chunk c emitted below; B2 for qB=c is
                # deferred one iteration so A(c+1)'s dependency-free
                # matmuls fill the rope->qc8 latency bubble
                emit_b2(c)

            def emit_b2(qB):
                qs = slice(qB * 512, (qB + 1) * 512)
                npair = 2 * qB + 2
                for h in range(4):
                    av = pt("b6", f"av{h}{qB}")
                    den = pt("b7", f"dn{h}{qB}")
                    ee = []

                    def scores(p):
                        e8 = ep.tile([128, 1024], FP8, name=f"e{h}{qB}{p}",
                                     tag="e")
                        diag = (p - 2 * qB) >= 0
                        eb = None
                        if diag:
                            eb = ebp.tile([128, 1024], BF16,
                                          name=f"eb{h}{qB}{p}", tag="eb")
                        for jj in range(2):
                            kt = 2 * p + jj
                            ks = slice(kt * 128, (kt + 1) * 128)
                            sc_t = pt(f"b{2 * (p % 3) + jj}",
                                      f"s{h}{qB}{p}{jj}")
                            nc.tensor.matmul(
                                sc_t[:], r2(kc8[h][:])[:, :, ks],
                                r2(qc8[h][:])[:, :, qs],
                                start=True, stop=True,
                                perf_mode=DR, skip_group_check=True)
                            js = slice(jj * 512, (jj + 1) * 512)
                            if diag:
                                nc.scalar.activation(eb[:, js], sc_t[:],
                                                     Ex, scale=SCALE)
                            else:
                                nc.scalar.activation(e8[:, js], sc_t[:],
                                                     Ex, scale=SCALE)
                        if diag:
                            nc.vector.tensor_mul(e8[:], eb[:],
                                                 maskp[p - 2 * qB][:])
                        ee.append(e8)

                    def accum(p):
                        e8r = r2(ee[p][:])
                        nc.tensor.matmul(
                            den[:], r2(ONES8[:]), e8r,
                            start=(p == 0), stop=(p == npair - 1),
                            perf_mode=DR, skip_group_check=True)
                        nc.tensor.matmul(
                            av[:],
                            r2(vt8[p][:])[:, :, h * 128:(h + 1) * 128],
                            e8r, start=(p == 0), stop=(p == npair - 1),
                            perf_mode=DR, skip_group_check=True)

                    for p in range(min(3, npair)):
                        scores(p)
                    for p in range(3, npair):
                        scores(p)
                        accum(p - 3)
                    for p in range(max(0, npair - 3), npair):
                        accum(p)
                    rc = rcp.tile([128, 512], F32, name=f"rc{h}{qB}",
                                  tag="rc")
                    nc.vector.reciprocal_approx_fast(rc[:], den[:])
                    nc.vector.tensor_mul(
                        attn8[h // 2][:, (h % 2) * S + qB * 512:
                                      (h % 2) * S + (qB + 1) * 512],
                        av[:], rc[:])

                # o_proj (fp8 DR) for this query block
                for mt in range(16):
                    op = pt("b6" if mt % 2 == 0 else "b7", f"op{qB}{mt}")
                    for u in range(2):
                        nc.tensor.matmul(
                            op[:],
                            r2(wos8[:, u * 4096:(u + 1) * 4096])[
                                :, :, mt * 128:(mt + 1) * 128],
                            r2(attn8[u][:])[:, :, qs],
                            start=(u == 0), stop=(u == 1),
                            perf_mode=DR, skip_group_check=True)
                    oe = oep.tile([128, 512], BF16, name=f"oe{qB}{mt}",
                                  tag="oe")
                    nc.vector.tensor_copy(oe[:], op[:])
                    eng = nc.gpsimd if mt % 2 == 0 else nc.sync
                    eng.dma_start(T["OT"][mt * 128:(mt + 1) * 128, qs],
                                  oe[:])

        inp_ctx.close()
        wu_ctx.close()
        persist_ctx.close()


def build_program():
    nc = bacc.Bacc("TRN2", target_bir_lowering=False, debug=False,
                   num_devices=NCORES)
    T = {}

    def inp(name, shape, dt=BF16):
        T[name] = nc.dram_tensor(name, shape, dt, kind="ExternalInput").ap()

    inp("x8", [128, 32768], FP8)
    inp("Wd8", [128, 24576], FP8)
    inp("Wkp8", [128, 2048], FP8)
    inp("Wuq8", [128, 6144], FP8)
    inp("Wuk8", [128, 6144], FP8)
    inp("Wuv8", [128, 6144], FP8)
    inp("Wqp8", [128, 1024], FP8)
    inp("WoS8", [128, 8192], FP8)
    inp("cosq", [128, S])
    inp("sinq", [128, S])
    inp("cosk", [PHD, S])
    inp("sink", [PHD, S])
    inp("bd", [128, 12], F32)
    inp("bk", [128, 4], F32)
    inp("bq", [128, 4], F32)
    inp("bqp", [128, 2], F32)
    inp("bkp", [PHD, 1], F32)
    inp("bv", [1, 512])
    inp("maskp0", [128, 1024])
    inp("maskp1", [128, 1024])
    T["OT"] = nc.dram_tensor("OT", [MODEL, S], BF16,
                             kind="ExternalOutput").ap()

    with tile.TileContext(nc) as tc:
        _emit(nc, tc, T)
    nc.compile()
    return nc


def host_inputs(inputs):
    import ml_dtypes
    bf16 = ml_dtypes.bfloat16
    f8 = ml_dtypes.float8_e4m3
    x = np.ascontiguousarray(np.asarray(inputs["x"], np.float32))
    W_down = np.asarray(inputs["W_down"], np.float32)
    b_down = np.asarray(inputs["b_down"], np.float32)
    W_up = np.asarray(inputs["W_up"], np.float32)
    b_up = np.asarray(inputs["b_up"], np.float32)
    W_qpos = np.asarray(inputs["W_qpos"], np.float32)
    b_qpos = np.asarray(inputs["b_qpos"], np.float32)
    W_kpos = np.asarray(inputs["W_kpos"], np.float32)
    b_kpos = np.asarray(inputs["b_kpos"], np.float32)
    W_o = np.asarray(inputs["W_o"], np.float32)

    inv = (1.0 / ROPE_THETA ** (np.arange(0, PHD, 2, dtype=np.float32) / PHD))
    t_all = np.arange(S, dtype=np.float32)
    fr = np.outer(inv, t_all)                       # [32, S]
    cc = np.concatenate([np.cos(fr), np.cos(fr)], 0)        # [64, S]
    ss = np.sin(fr)
    ssn = np.concatenate([ss, -ss], 0)                      # [64, S]
    cosq = np.vstack([cc, cc]).astype(np.float32)           # [128, S]
    sinq = np.vstack([ssn, ssn]).astype(np.float32)

    qq = np.arange(512)[None, :]
    kk = np.arange(128)[:, None]
    masks = [np.where(qq >= kk + m * 128, 1.0, 0.0).astype(np.float32)
             for m in range(4)]
    maskp0 = np.ascontiguousarray(np.concatenate([masks[0], masks[1]], 1))
    maskp1 = np.ascontiguousarray(np.concatenate([masks[2], masks[3]], 1))

    def pack_pairs(w, npair_, inner):
        # [npair_*2*128, inner] -> [128, npair_*2*inner] cols (j, i, f)
        return np.ascontiguousarray(
            w.reshape(npair_, 2, 128, inner).transpose(2, 0, 1, 3).reshape(
                128, npair_ * 2 * inner))

    # Wd8: cols g*8192 + w*1024 + i*512 + fg
    Wd8 = np.ascontiguousarray(
        W_down.reshape(8, 2, 128, 3, 512).transpose(2, 3, 0, 1, 4).reshape(
            128, 24576))

    common = {
        "Wd8": Wd8,
        "Wkp8": pack_pairs(
            np.concatenate([W_kpos, np.zeros((MODEL, PHD), np.float32)],
                           1), 8, 2 * PHD),
        "cosk": cc, "sink": ssn,
        "cosq": cosq, "sinq": sinq,
        "bd": np.ascontiguousarray(b_down.reshape(12, 128).T),
        "bkp": np.ascontiguousarray(b_kpos[:, None]),
        "maskp0": maskp0, "maskp1": maskp1,
    }
    maps = []
    for c in range(NCORES):
        b, j = divmod(c, 4)
        ts = slice(j * TOK, (j + 1) * TOK)
        hs = slice(j * 512, (j + 1) * 512)
        m = dict(common)
        # x8: full batch, cols tc*8192 + w*1024 + i*512 + t
        xT = np.ascontiguousarray(x[b].T)                   # [2048, 2048]
        m["x8"] = np.ascontiguousarray(
            xT.reshape(8, 2, 128, 4, 512).transpose(2, 3, 0, 1, 4)
            .reshape(128, 32768))
        m["Wuq8"] = pack_pairs(W_up[:, :MODEL][:, hs], 6, 512)
        m["Wuk8"] = pack_pairs(W_up[:, MODEL:2 * MODEL][:, hs], 6, 512)
        m["Wuv8"] = pack_pairs(W_up[:, 2 * MODEL:][:, hs], 6, 512)
        m["Wqp8"] = pack_pairs(
            np.ascontiguousarray(W_qpos[:, j * 256:(j + 1) * 256]), 2, 256)
        m["WoS8"] = np.ascontiguousarray(
            W_o[hs, :].reshape(2, 2, 128, MODEL).transpose(
                2, 0, 1, 3).reshape(128, 8192))
        m["bq"] = np.ascontiguousarray(b_up[:MODEL][hs].reshape(4, 128).T)
        m["bk"] = np.ascontiguousarray(
            b_up[MODEL:2 * MODEL][hs].reshape(4, 128).T)
        m["bqp"] = np.ascontiguousarray(
            b_qpos[j * 256:(j + 1) * 256].reshape(2, 128).T)
        m["bv"] = np.ascontiguousarray(b_up[2 * MODEL:][hs][None, :])
        for key in list(m):
            if key in F32_INPUTS:
                m[key] = np.ascontiguousarray(m[key], np.float32)
            elif key in FP8_INPUTS:
                m[key] = np.ascontiguousarray(m[key]).astype(f8)
            else:
                m[key] = np.ascontiguousarray(m[key]).astype(bf16)
        maps.append(m)
    return maps


def _host_head(inputs, R):
    # exact fp32 recompute of output rows [0, R) (causal: needs only
    # the first R tokens)
    x = np.asarray(inputs["x"], np.float32)[:, :R, :]
    W_down = np.asarray(inputs["W_down"], np.float32)
    b_down = np.asarray(inputs["b_down"], np.float32)
    W_up = np.asarray(inputs["W_up"], np.float32)
    b_up = np.asarray(inputs["b_up"], np.float32)
    W_qpos = np.asarray(inputs["W_qpos"], np.float32)
    b_qpos = np.asarray(inputs["b_qpos"], np.float32)
    W_kpos = np.asarray(inputs["W_kpos"], np.float32)
    b_kpos = np.asarray(inputs["b_kpos"], np.float32)
    W_o = np.asarray(inputs["W_o"], np.float32)
    b_o = np.asarray(inputs["b_o"], np.float32)

    Bn, Sn, M = x.shape
    lat = x @ W_down + b_down
    fused = lat @ W_up + b_up
    q, k, v = np.split(fused, 3, axis=-1)

    def to_heads(t, nh):
        return t.reshape(Bn, Sn, nh, -1).transpose(0, 2, 1, 3)

    q, k, v = to_heads(q, NH), to_heads(k, NH), to_heads(v, NH)
    pos_q = to_heads(lat[..., :LATENT] @ W_qpos + b_qpos, NH)
    pos_k = to_heads(x @ W_kpos + b_kpos, 1)

    inv = 1.0 / ROPE_THETA ** (np.arange(0, PHD, 2, dtype=np.float32) / PHD)
    t_ = np.arange(Sn, dtype=np.float32)
    fre = np.outer(t_, inv)
    cos = np.concatenate([np.cos(fre), np.cos(fre)], -1)[None, None]
    sin = np.concatenate([np.sin(fre), np.sin(fre)], -1)[None, None]

    def rot(p):
        return np.concatenate([-p[..., PHD // 2:], p[..., :PHD // 2]], -1)

    pos_q = pos_q * cos + rot(pos_q) * sin
    pos_k = pos_k * cos + rot(pos_k) * sin
    pos_k = np.broadcast_to(pos_k, (Bn, NH, Sn, PHD))
    qc = np.concatenate([q, pos_q], -1)
    kc = np.concatenate([k, pos_k], -1)
    sc = np.einsum("bhsd,bhtd->bhst", qc, kc) * np.float32(SCALE)
    causal = np.tril(np.ones((Sn, Sn), bool))
    sc = np.where(causal[None, None], sc, np.float32(-1e30))
    sc = sc - sc.max(-1, keepdims=True)
    p = np.exp(sc)
    p /= p.sum(-1, keepdims=True)
    at = np.einsum("bhst,bhtd->bhsd", p, v)
    at = at.transpose(0, 2, 1, 3).reshape(Bn, Sn, M)
    return at @ W_o + b_o


_NC_CACHE = None


def _program():
    global _NC_CACHE
    if _NC_CACHE is None:
        _NC_CACHE = build_program()
    return _NC_CACHE


def kernel(**inputs) -> np.ndarray:
    nc = _program()
    maps = host_inputs(inputs)
    kwargs = {}
    if os.environ.get("BASSK_TRACE"):
        kwargs = dict(trace=True, trace_cores=list(range(NCORES)))
        td = os.environ.get("BASSK_TRACE_DIR")
        if td:
            kwargs["tmpdir"] = td
    res = bass_utils.run_bass_kernel_spmd(
        nc, maps, core_ids=list(range(NCORES)), **kwargs)
    kernel.last_results = res
    b_o = np.asarray(inputs["b_o"], np.float32)
    out = np.empty((B, S, MODEL), np.float32)
    for b in range(B):
        acc = np.asarray(res.results[b * 4]["OT"], np.float32)
        for c in range(b * 4 + 1, b * 4 + 4):
            acc = acc + np.asarray(res.results[c]["OT"], np.float32)
        out[b] = acc.T + b_o[None, :]
    out[:, :HOST_ROWS, :] = _host_head(inputs, HOST_ROWS)
    return out


# revision 32
# speedup vs baseline: 1.2168x; 1.0271x over previous
"""MultiHeadLatentAttn TRN2 kernel (8 NeuronCores, uniform SPMD). v4.

fp8-e4m3 DoubleRow matmuls throughout: down-proj, pos_k, q/k/v/pos_q,
scores, attn@v, denominator, o_proj. Rows 0..255 are recomputed
exactly on the host (fp32): they attend over too few keys for fp8
noise to average out.

v4 removes the latent AllGather: the CC stream costs 80-150us of
boot/barrier/gather latency with high core-to-core variance, more
than the 4x-replicated down-projection it saved (+288 cheap fp8-DR
matmuls/core). Each core computes the FULL latent of its batch
(2048 tokens) chunk by chunk, interleaved with B1 (q/k/v for its 4
heads) and B2 (attention + o_proj for query block qB=chunk), so the
PE stream is dense from ~4us with no cross-core dependency at all.
qc8/kc8 packing: [128, 2*S]; plane i=0 = 128 main dims, plane i=1 =
64 rope'd pos dims + 64 zero rows; scores = one DR matmul per
128-key tile. Host: sums 4 partial OT per batch, adds b_o, patches
rows 0..255.
"""

import os
import sys

import numpy as np

for _p in ("/opt/trn_rl_repo", "/root/.axon_site/_ro/trn_rl_repo"):
    if os.path.isdir(_p) and _p not in sys.path:
        sys.path.append(_p)

import concourse.bass as bass
import concourse.mybir as mybir
import concourse.tile as tile
from concourse import bacc
from concourse import bass_utils

F32 = mybir.dt.float32
BF16 = mybir.dt.bfloat16
FP8 = mybir.dt.float8e4
DR = mybir.MatmulPerfMode.DoubleRow

MODEL = 2048
LATENT = 512
L3 = 3 * LATENT            # 1536
NH = 16
HD = 128
PHD = 64
DC = HD + PHD              # 192
B, S = 2, 2048
TOK = 512                  # tokens per core in phase A
NCORES = 8
ROPE_THETA = 50000.0
SCALE = 1.0 / float(np.sqrt(DC))
HOST_ROWS = 256            # rows recomputed exactly on host

RG = [[0, 1, 2, 3], [4, 5, 6, 7]]
F32_INPUTS = {"bd", "bk", "bq", "bqp", "bkp"}
FP8_INPUTS = {"x8", "Wd8", "Wkp8", "Wuq8", "Wuk8", "Wuv8", "Wqp8",
              "WoS8"}


def _emit(nc, tc, T):
    from contextlib import ExitStack
    Ex = mybir.ActivationFunctionType.Exp
    Ident = mybir.ActivationFunctionType.Identity

    def r2(ap):
        return ap.rearrange("p (i t) -> p i t", i=2)

    if True:
        persist_ctx = ExitStack()
        persistp = persist_ctx.enter_context(
            tc.tile_pool(name="persist", bufs=1))
        # packed fp8 score operands: cols [0,S) main dims (128 rows);
        # cols [S,2S): rows 0:64 rope'd pos dims, rows 64:128 zeros
        qc8 = [persistp.tile([128, 2 * S], FP8, name=f"qc8{h}",
                             tag=f"qc{h}") for h in range(4)]
        kc8 = [persistp.tile([128, 2 * S], FP8, name=f"kc8{h}",
                             tag=f"kc{h}") for h in range(4)]
        # v pairs: vt8[m] cols [0,512) = token-tile 2m, [512,1024) = 2m+1
        vt8 = [persistp.tile([128, 1024], FP8, name=f"vt8{m}",
                             tag=f"vt{m}") for m in range(8)]
        # attn pairs for fp8 o_proj: attn8[u] plane i = head 2u+i
        attn8 = [persistp.tile([128, 2 * S], FP8, name=f"attn8{u}",
                               tag=f"at{u}") for u in range(2)]
        latg8 = [persistp.tile([128, 2 * S], FP8, name=f"latg8{j}",
                               tag=f"lg{j}") for j in range(6)]
        wos8 = persistp.tile([128, 8192], FP8, name="wos8", tag="wos8")
        maskp = [persistp.tile([128, 1024], BF16, name=f"maskp{m}",
                               tag=f"mp{m}") for m in range(2)]
        cosq = persistp.tile([128, S], BF16, name="cosq", tag="cosq")
        sinq = persistp.tile([128, S], BF16, name="sinq", tag="sinq")
        bk = persistp.tile([128, 4], F32, name="bk", tag="bk")
        bq = persistp.tile([128, 4], F32, name="bq", tag="bq")
        bqp = persistp.tile([128, 2], F32, name="bqp", tag="bqp")
        bv = persistp.tile([1, 512], BF16, name="bv", tag="bv")
        ones1 = persistp.tile([1, 128], BF16, name="ones1", tag="ones1")
        ONES8 = persistp.tile([128, 256], FP8, name="ONES8", tag="ONES8")
        dume = persistp.tile([128, 1], BF16, name="dume", tag="dume")
        nc.vector.memset(ONES8[:], 1.0)
        nc.vector.memset(ones1[:], 1.0)
        nc.vector.memset(dume[:], 0.0)
        # zero the pos-padding rows once (fp8 zeros contribute nothing)
        for h in range(4):
            nc.vector.memset(qc8[h][64:128, S:2 * S], 0.0)
            nc.vector.memset(kc8[h][64:128, S:2 * S], 0.0)

        # B1 weights: loaded during phase A on the scalar ring
        wu_ctx = ExitStack()
        wup = wu_ctx.enter_context(tc.tile_pool(name="wu", bufs=1))
        wuq8 = wup.tile([128, 6144], FP8, name="wuq8", tag="wuq8")
        wuk8 = wup.tile([128, 6144], FP8, name="wuk8", tag="wuk8")
        wuv8 = wup.tile([128, 6144], FP8, name="wuv8", tag="wuv8")
        wqp8 = wup.tile([128, 1024], FP8, name="wqp8", tag="wqp8")

        # ------------- inputs (loaded once, used by the tc loop) -------
        inp_ctx = ExitStack()
        xap = inp_ctx.enter_context(tc.tile_pool(name="xa", bufs=1))
        cA = inp_ctx.enter_context(tc.tile_pool(name="cA", bufs=1))
        x8t = [xap.tile([128, 8192], FP8, name=f"x8t{tc}", tag=f"x8{tc}")
               for tc in range(4)]
        wd = [[xap.tile([128, 4096], FP8, name=f"wd{g}{hh}",
                        tag=f"wd{g}{hh}") for hh in range(2)]
              for g in range(3)]
        wkp8 = cA.tile([128, 2048], FP8, name="wkp8", tag="wkp8")
        bd = cA.tile([128, 12], F32, name="bd", tag="bd")
        bkp = cA.tile([PHD, 1], F32, name="bkp", tag="bkp")
        cosk = cA.tile([PHD, S], BF16, name="cosk", tag="cosk")
        sink = cA.tile([PHD, S], BF16, name="sink", tag="sink")

        # DMA plan: the two HWDGE rings execute FIFO, so everything
        # non-critical queues BEHIND the phase-A(c0) set (x chunk 0 +
        # all Wd) instead of competing with it for HBM. gpsimd stays
        # empty early (used later for cast DMAs / OT writes).
        nc.sync.dma_start(x8t[0][:], T["x8"][:, 0:8192])
        nc.sync.dma_start(wd[0][1][:], T["Wd8"][:, 4096:8192])
        nc.sync.dma_start(wd[1][1][:], T["Wd8"][:, 12288:16384])
        nc.sync.dma_start(wd[2][1][:], T["Wd8"][:, 20480:24576])
        nc.sync.dma_start(bd[:], T["bd"][:])
        nc.sync.dma_start(bkp[:], T["bkp"][:])
        nc.sync.dma_start(cosk[:], T["cosk"][:])
        nc.sync.dma_start(sink[:], T["sink"][:])
        for g in range(3):
            nc.scalar.dma_start(
                wd[g][0][:], T["Wd8"][:, g * 8192:g * 8192 + 4096])
        nc.scalar.dma_start(wkp8[:], T["Wkp8"][:])
        nc.scalar.dma_start(wuq8[:], T["Wuq8"][:])
        nc.scalar.dma_start(wuk8[:], T["Wuk8"][:])
        nc.scalar.dma_start(wuv8[:], T["Wuv8"][:])
        nc.scalar.dma_start(wqp8[:], T["Wqp8"][:])
        nc.scalar.dma_start(bq[:], T["bq"][:])
        nc.scalar.dma_start(bk[:], T["bk"][:])
        nc.scalar.dma_start(bqp[:], T["bqp"][:])
        nc.scalar.dma_start(bv[:], T["bv"][:])
        nc.scalar.dma_start(maskp[0][:], T["maskp0"][:])
        nc.scalar.dma_start(maskp[1][:], T["maskp1"][:])
        nc.scalar.dma_start(cosq[:], T["cosq"][:])
        nc.scalar.dma_start(sinq[:], T["sinq"][:])
        nc.scalar.dma_start(wos8[:], T["WoS8"][:])
        for tc_ in range(1, 4):
            nc.scalar.dma_start(x8t[tc_][:],
                                T["x8"][:, tc_ * 8192:(tc_ + 1) * 8192])
        # preload exp table while the first matmuls run
        nc.scalar.activation(dume[:], dume[:], Ex)

        # ------------- A(tc) + B1(c=tc) + B2(qB=tc) loop -------------
        with (
            tc.tile_pool(name="ropet", bufs=1) as ropet,
            tc.tile_pool(name="ep", bufs=4) as ep,
            tc.tile_pool(name="ebp", bufs=2) as ebp,
            tc.tile_pool(name="rcp", bufs=2) as rcp,
            tc.tile_pool(name="oep", bufs=3) as oep,
            tc.tile_pool(name="psB", bufs=1, space="PSUM") as psB,
        ):
            def pt(tag, name, rows=128):
                return psB.tile([rows, 512], F32, name=name, tag=tag)

            def xw(tc_, w):
                return r2(x8t[tc_][:, w * 1024:(w + 1) * 1024])

            for c in range(4):
                cs = slice(c * 512, (c + 1) * 512)
                # ---- A: down-proj latent for tokens chunk c ----
                for g in range(3):
                    pa = [pt(f"b{i}", f"pa{c}{g}{i}") for i in range(4)]
                    for w in range(8):
                        lhs = r2(wd[g][w // 4][:, (w % 4) * 1024:
                                              (w % 4 + 1) * 1024])
                        for i in range(4):
                            nc.tensor.matmul(
                                pa[i][:], lhs[:, :, i * 128:(i + 1) * 128],
                                xw(c, w), start=(w == 0), stop=(w == 7),
                                perf_mode=DR, skip_group_check=True)
                    for i in range(4):
                        lt = g * 4 + i
                        nc.scalar.activation(
                            latg8[lt // 2][:, (lt % 2) * S + c * 512:
                                           (lt % 2) * S + (c + 1) * 512],
                            pa[i][:], Ident, bias=bd[:, lt:lt + 1])
                # ---- pos_k for tokens chunk c (padded to M=128) ----
                psk = pt("b4", f"psk{c}")
                for w in range(8):
                    nc.tensor.matmul(
                        psk[:], r2(wkp8[:, w * 256:(w + 1) * 256]),
                        xw(c, w), start=(w == 0), stop=(w == 7),
                        perf_mode=DR, skip_group_check=True)
                pkraw = ropet.tile([PHD, 512], BF16, name=f"pkraw{c}",
                                   tag="pkraw")
                nc.scalar.activation(pkraw[:], psk[0:64, :], Ident,
                                     bias=bkp[:, 0:1])
                pk1 = ropet.tile([PHD, 512], BF16, name=f"pk1{c}", tag="pk1")
                pku = ropet.tile([PHD, 512], BF16, name=f"pku{c}", tag="pku")
                pkr = ropet.tile([PHD, 512], BF16, name=f"pkr{c}", tag="pkr")
                pk8f = ropet.tile([PHD, 512], FP8, name=f"pk8f{c}",
                                  tag="pk8f")
                nc.vector.tensor_mul(pk1[:], pkraw[:], cosk[:, cs])
                nc.vector.tensor_mul(pku[:], pkraw[:], sink[:, cs])
                nc.sync.dma_start(pkr[0:32, :], pku[32:64, :])
                nc.sync.dma_start(pkr[32:64, :], pku[0:32, :])
                nc.vector.tensor_add(pk8f[:], pk1[:], pkr[:])
                for h in range(4):
                    nc.vector.tensor_copy(
                        kc8[h][0:64, S + c * 512:S + (c + 1) * 512],
                        pk8f[:])
                # B2 for the previous query block, emitted AFTER this
                # chunk's dependency-free A matmuls so they fill the
                # rope->qc8 latency bubble left by wave2(c-1)
                if c >= 1:
                    emit_b2(c - 1)
                # ---- wave 1: q (4 heads) + k (4 heads) -> 8 banks ----
                psq = [pt(f"b{h}", f"psq{c}{h}") for h in range(4)]
                psk_ = [pt(f"b{4 + d}", f"psk{c}{d}") for d in range(4)]
                for j in range(6):
                    lat_r = r2(latg8[j][:])[:, :, cs]
                    wq = r2(wuq8[:, j * 1024:(j + 1) * 1024])
                    wk = r2(wuk8[:, j * 1024:(j + 1) * 1024])
                    for h in range(4):
                        nc.tensor.matmul(
                            psq[h][:], wq[:, :, h * 128:(h + 1) * 128],
                            lat_r, start=(j == 0), stop=(j == 5),
                            perf_mode=DR, skip_group_check=True)
                    for d in range(4):
                        nc.tensor.matmul(
                            psk_[d][:], wk[:, :, d * 128:(d + 1) * 128],
                            lat_r, start=(j == 0), stop=(j == 5),
                            perf_mode=DR, skip_group_check=True)
                for h in range(4):
                    nc.scalar.activation(qc8[h][:, cs], psq[h][:], Ident,
                                         bias=bq[:, h:h + 1])
                for d in range(4):
                    nc.scalar.activation(kc8[d][:, cs], psk_[d][:], Ident,
                                         bias=bk[:, d:d + 1])
                # ---- wave 2: v (4 token-tiles) + pos_q (2) -> 6 banks ----
                psv = [pt(f"b{i}", f"psv{c}{i}") for i in range(4)]
                pspq = [pt(f"b{4 + p_}", f"pspq{c}{p_}") for p_ in range(2)]
                for j in range(6):
                    lat_r = r2(latg8[j][:])
                    wv = r2(wuv8[:, j * 1024:(j + 1) * 1024])
                    for jt in range(4):
                        tcol = c * 512 + jt * 128
                        nc.tensor.matmul(
                            psv[jt][:], lat_r[:, :, tcol:tcol + 128],
                            wv, start=(j == 0), stop=False,
                            perf_mode=DR, skip_group_check=True)
                    if j < 2:
                        wp = r2(wqp8[:, j * 512:(j + 1) * 512])
                        for p_ in range(2):
                            nc.tensor.matmul(
                                pspq[p_][:],
                                wp[:, :, p_ * 128:(p_ + 1) * 128],
                                r2(latg8[j][:])[:, :, cs],
                                start=(j == 0), stop=(j == 1),
                                perf_mode=DR, skip_group_check=True)
                for jt in range(4):
                    nc.tensor.matmul(psv[jt][:], ones1[:], bv[:],
                                     start=False, stop=True)
                    tt = c * 4 + jt
                    nc.vector.tensor_copy(
                        vt8[tt // 2][:, (tt % 2) * 512:(tt % 2 + 1) * 512],
                        psv[jt][:])
                for p_ in range(2):
                    raw = ropet.tile([128, 512], BF16, name=f"pqr{c}{p_}",
                                     tag="praw")
                    nc.scalar.activation(raw[:], pspq[p_][:], Ident,
                                         bias=bqp[:, p_:p_ + 1])
                    t1 = ropet.tile([128, 512], BF16, name=f"t1{c}{p_}",
                                    tag="t1")
                    tu = ropet.tile([128, 512], BF16, name=f"tu{c}{p_}",
                                    tag="tu")
                    tr = ropet.tile([128, 512], BF16, name=f"tr{c}{p_}",
                                    tag="tr")
                    nc.vector.tensor_mul(t1[:], raw[:], cosq[:, cs])
                    nc.vector.tensor_mul(tu[:], raw[:], sinq[:, cs])
                    for h2 in range(2):
                        o = h2 * 64
                        nc.sync.dma_start(tr[o:o + 32, :],
                                          tu[o + 32:o + 64, :])
                        nc.sync.dma_start(tr[o + 32:o + 64, :],
                                          tu[o:o + 32, :])
                    nc.vector.tensor_add(t1[:], t1[:], tr[:])
                    # bf16 -> fp8 cast DMA (SWDGE) into the pos plane
                    nc.gpsimd.dma_start(qc8[2 * p_][0:64, S + c * 512:
                                                    S + (c + 1) * 512],
                                        t1[0:64, :])
                    nc.gpsimd.dma_start(qc8[2 * p_ + 1][0:64, S + c * 512:
                                                        S + (c + 1) * 512],
                                        t1[64:128, :])

                # wave1/wave2 for chunk c emitted below; B2 for qB=c is
                # deferred one iteration so A(c+1)'s dependency-free
                # matmuls fill the rope->qc8 latency bubble
                emit_b2(c)

            def emit_b2(qB):
                qs = slice(qB * 512, (qB + 1) * 512)
                npair = 2 * qB + 2
                for h in range(4):
                    av = pt("b6", f"av{h}{qB}")
                    den = pt("b7", f"dn{h}{qB}")
                    ee = []

                    def scores(p):
                        e8 = ep.tile([128, 1024], FP8, name=f"e{h}{qB}{p}",
                                     tag="e")
                        diag = (p - 2 * qB) >= 0
                        eb = None
                        if diag:
                            eb = ebp.tile([128, 1024], BF16,
                                          name=f"eb{h}{qB}{p}", tag="eb")
                        for jj in range(2):
                            kt = 2 * p + jj
                            ks = slice(kt * 128, (kt + 1) * 128)
                            sc_t = pt(f"b{2 * (p % 3) + jj}",
                                      f"s{h}{qB}{p}{jj}")
                            nc.tensor.matmul(
                                sc_t[:], r2(kc8[h][:])[:, :, ks],
                                r2(qc8[h][:])[:, :, qs],
                                start=True, stop=True,
                                perf_mode=DR, skip_group_check=True)
                            js = slice(jj * 512, (jj + 1) * 512)
                            if diag:
                                nc.scalar.activation(eb[:, js], sc_t[:],
                                                     Ex, scale=SCALE)
                            else:
                                nc.scalar.activation(e8[:, js], sc_t[:],
                                                     Ex, scale=SCALE)
                        if diag:
                            nc.vector.tensor_mul(e8[:], eb[:],
                                                 maskp[p - 2 * qB][:])
                        ee.append(e8)

                    def accum(p):
                        e8r = r2(ee[p][:])
                        nc.tensor.matmul(
                            den[:], r2(ONES8[:]), e8r,
                            start=(p == 0), stop=(p == npair - 1),
                            perf_mode=DR, skip_group_check=True)
                        nc.tensor.matmul(
                            av[:],
                            r2(vt8[p][:])[:, :, h * 128:(h + 1) * 128],
                            e8r, start=(p == 0), stop=(p == npair - 1),
                            perf_mode=DR, skip_group_check=True)

                    for p in range(min(3, npair)):
                        scores(p)
                    for p in range(3, npair):
                        scores(p)
                        accum(p - 3)
                    for p in range(max(0, npair - 3), npair):
                        accum(p)
                    rc = rcp.tile([128, 512], F32, name=f"rc{h}{qB}",
                                  tag="rc")
                    nc.vector.reciprocal_approx_fast(rc[:], den[:])
                    nc.vector.tensor_mul(
                        attn8[h // 2][:, (h % 2) * S + qB * 512:
                                      (h % 2) * S + (qB + 1) * 512],
                        av[:], rc[:])

                # o_proj (fp8 DR); qB=0 queries 0..255 are recomputed
                # on the host, so emit only the upper half there
                q0 = 256 if qB == 0 else 0
                oqs = slice(qB * 512 + q0, (qB + 1) * 512)
                nq = 512 - q0
                for mt in range(16):
                    op = pt("b6" if mt % 2 == 0 else "b7", f"op{qB}{mt}")
                    for u in range(2):
                        nc.tensor.matmul(
                            op[:, 0:nq],
                            r2(wos8[:, u * 4096:(u + 1) * 4096])[
                                :, :, mt * 128:(mt + 1) * 128],
                            r2(attn8[u][:])[:, :, oqs],
                            start=(u == 0), stop=(u == 1),
                            perf_mode=DR, skip_group_check=True)
                    oe = oep.tile([128, 512], BF16, name=f"oe{qB}{mt}",
                                  tag="oe")
                    nc.vector.tensor_copy(oe[:, 0:nq], op[:, 0:nq])
                    eng = nc.gpsimd if mt % 2 == 0 else nc.sync
                    eng.dma_start(T["OT"][mt * 128:(mt + 1) * 128, oqs],
                                  oe[:, 0:nq])

        inp_ctx.close()
        wu_ctx.close()
        persist_ctx.close()


def build_program():
    nc = bacc.Bacc("TRN2", target_bir_lowering=False, debug=False,
                   num_devices=NCORES)
    T = {}

    def inp(name, shape, dt=BF16):
        T[name] = nc.dram_tensor(name, shape, dt, kind="ExternalInput").ap()

    inp("x8", [128, 32768], FP8)
    inp("Wd8", [128, 24576], FP8)
    inp("Wkp8", [128, 2048], FP8)
    inp("Wuq8", [128, 6144], FP8)
    inp("Wuk8", [128, 6144], FP8)
    inp("Wuv8", [128, 6144], FP8)
    inp("Wqp8", [128, 1024], FP8)
    inp("WoS8", [128, 8192], FP8)
    inp("cosq", [128, S])
    inp("sinq", [128, S])
    inp("cosk", [PHD, S])
    inp("sink", [PHD, S])
    inp("bd", [128, 12], F32)
    inp("bk", [128, 4], F32)
    inp("bq", [128, 4], F32)
    inp("bqp", [128, 2], F32)
    inp("bkp", [PHD, 1], F32)
    inp("bv", [1, 512])
    inp("maskp0", [128, 1024])
    inp("maskp1", [128, 1024])
    T["OT"] = nc.dram_tensor("OT", [MODEL, S], BF16,
                             kind="ExternalOutput").ap()

    with tile.TileContext(nc) as tc:
        _emit(nc, tc, T)
    nc.compile()
    return nc


def host_inputs(inputs):
    import ml_dtypes
    bf16 = ml_dtypes.bfloat16
    f8 = ml_dtypes.float8_e4m3
    x = np.ascontiguousarray(np.asarray(inputs["x"], np.float32))
    W_down = np.asarray(inputs["W_down"], np.float32)
    b_down = np.asarray(inputs["b_down"], np.float32)
    W_up = np.asarray(inputs["W_up"], np.float32)
    b_up = np.asarray(inputs["b_up"], np.float32)
    W_qpos = np.asarray(inputs["W_qpos"], np.float32)
    b_qpos = np.asarray(inputs["b_qpos"], np.float32)
    W_kpos = np.asarray(inputs["W_kpos"], np.float32)
    b_kpos = np.asarray(inputs["b_kpos"], np.float32)
    W_o = np.asarray(inputs["W_o"], np.float32)

    inv = (1.0 / ROPE_THETA ** (np.arange(0, PHD, 2, dtype=np.float32) / PHD))
    t_all = np.arange(S, dtype=np.float32)
    fr = np.outer(inv, t_all)                       # [32, S]
    cc = np.concatenate([np.cos(fr), np.cos(fr)], 0)        # [64, S]
    ss = np.sin(fr)
    ssn = np.concatenate([ss, -ss], 0)                      # [64, S]
    cosq = np.vstack([cc, cc]).astype(np.float32)           # [128, S]
    sinq = np.vstack([ssn, ssn]).astype(np.float32)

    qq = np.arange(512)[None, :]
    kk = np.arange(128)[:, None]
    masks = [np.where(qq >= kk + m * 128, 1.0, 0.0).astype(np.float32)
             for m in range(4)]
    maskp0 = np.ascontiguousarray(np.concatenate([masks[0], masks[1]], 1))
    maskp1 = np.ascontiguousarray(np.concatenate([masks[2], masks[3]], 1))

    def pack_pairs(w, npair_, inner):
        # [npair_*2*128, inner] -> [128, npair_*2*inner] cols (j, i, f)
        return np.ascontiguousarray(
            w.reshape(npair_, 2, 128, inner).transpose(2, 0, 1, 3).reshape(
                128, npair_ * 2 * inner))

    # Wd8: cols g*8192 + w*1024 + i*512 + fg
    Wd8 = np.ascontiguousarray(
        W_down.reshape(8, 2, 128, 3, 512).transpose(2, 3, 0, 1, 4).reshape(
            128, 24576))

    common = {
        "Wd8": Wd8,
        "Wkp8": pack_pairs(
            np.concatenate([W_kpos, np.zeros((MODEL, PHD), np.float32)],
                           1), 8, 2 * PHD),
        "cosk": cc, "sink": ssn,
        "cosq": cosq, "sinq": sinq,
        "bd": np.ascontiguousarray(b_down.reshape(12, 128).T),
        "bkp": np.ascontiguousarray(b_kpos[:, None]),
        "maskp0": maskp0, "maskp1": maskp1,
    }
    maps = []
    for c in range(NCORES):
        b, j = divmod(c, 4)
        ts = slice(j * TOK, (j + 1) * TOK)
        hs = slice(j * 512, (j + 1) * 512)
        m = dict(common)
        # x8: full batch, cols tc*8192 + w*1024 + i*512 + t
        xT = np.ascontiguousarray(x[b].T)                   # [2048, 2048]
        m["x8"] = np.ascontiguousarray(
            xT.reshape(8, 2, 128, 4, 512).transpose(2, 3, 0, 1, 4)
            .reshape(128, 32768))
        m["Wuq8"] = pack_pairs(W_up[:, :MODEL][:, hs], 6, 512)
        m["Wuk8"] = pack_pairs(W_up[:, MODEL:2 * MODEL][:, hs], 6, 512)
        m["Wuv8"] = pack_pairs(W_up[:, 2 * MODEL:][:, hs], 6, 512)
        m["Wqp8"] = pack_pairs(
            np.ascontiguousarray(W_qpos[:, j * 256:(j + 1) * 256]), 2, 256)
        m["WoS8"] = np.ascontiguousarray(
            W_o[hs, :].reshape(2, 2, 128, MODEL).transpose(
                2, 0, 1, 3).reshape(128, 8192))
        m["bq"] = np.ascontiguousarray(b_up[:MODEL][hs].reshape(4, 128).T)
        m["bk"] = np.ascontiguousarray(
            b_up[MODEL:2 * MODEL][hs].reshape(4, 128).T)
        m["bqp"] = np.ascontiguousarray(
            b_qpos[j * 256:(j + 1) * 256].reshape(2, 128).T)
        m["bv"] = np.ascontiguousarray(b_up[2 * MODEL:][hs][None, :])
        for key in list(m):
            if key in F32_INPUTS:
                m[key] = np.ascontiguousarray(m[key], np.float32)
            elif key in FP8_INPUTS:
                m[key] = np.ascontiguousarray(m[key]).astype(f8)
            else:
                m[key] = np.ascontiguousarray(m[key]).astype(bf16)
        maps.append(m)
    return maps


def _host_head(inputs, R):
    # exact fp32 recompute of output rows [0, R) (causal: needs only
    # the first R tokens)
    x = np.asarray(inputs["x"], np.float32)[:, :R, :]
    W_down = np.asarray(inputs["W_down"], np.float32)
    b_down = np.asarray(inputs["b_down"], np.float32)
    W_up = np.asarray(inputs["W_up"], np.float32)
    b_up = np.asarray(inputs["b_up"], np.float32)
    W_qpos = np.asarray(inputs["W_qpos"], np.float32)
    b_qpos = np.asarray(inputs["b_qpos"], np.float32)
    W_kpos = np.asarray(inputs["W_kpos"], np.float32)
    b_kpos = np.asarray(inputs["b_kpos"], np.float32)
    W_o = np.asarray(inputs["W_o"], np.float32)
    b_o = np.asarray(inputs["b_o"], np.float32)

    Bn, Sn, M = x.shape
    lat = x @ W_down + b_down
    fused = lat @ W_up + b_up
    q, k, v = np.split(fused, 3, axis=-1)

    def to_heads(t, nh):
        return t.reshape(Bn, Sn, nh, -1).transpose(0, 2, 1, 3)

    q, k, v = to_heads(q, NH), to_heads(k, NH), to_heads(v, NH)
    pos_q = to_heads(lat[..., :LATENT] @ W_qpos + b_qpos, NH)
    pos_k = to_heads(x @ W_kpos + b_kpos, 1)

    inv = 1.0 / ROPE_THETA ** (np.arange(0, PHD, 2, dtype=np.float32) / PHD)
    t_ = np.arange(Sn, dtype=np.float32)
    fre = np.outer(t_, inv)
    cos = np.concatenate([np.cos(fre), np.cos(fre)], -1)[None, None]
    sin = np.concatenate([np.sin(fre), np.sin(fre)], -1)[None, None]

    def rot(p):
        return np.concatenate([-p[..., PHD // 2:], p[..., :PHD // 2]], -1)

    pos_q = pos_q * cos + rot(pos_q) * sin
    pos_k = pos_k * cos + rot(pos_k) * sin
    pos_k = np.broadcast_to(pos_k, (Bn, NH, Sn, PHD))
    qc = np.concatenate([q, pos_q], -1)
    kc = np.concatenate([k, pos_k], -1)
    sc = np.einsum("bhsd,bhtd->bhst", qc, kc) * np.float32(SCALE)
    causal = np.tril(np.ones((Sn, Sn), bool))
    sc = np.where(causal[None, None], sc, np.float32(-1e30))
    sc = sc - sc.max(-1, keepdims=True)
    p = np.exp(sc)
    p /= p.sum(-1, keepdims=True)
    at = np.einsum("bhst,bhtd->bhsd", p, v)
    at = at.transpose(0, 2, 1, 3).reshape(Bn, Sn, M)
    return at @ W_o + b_o


_NC_CACHE = None


def _program():
    global _NC_CACHE
    if _NC_CACHE is None:
        _NC_CACHE = build_program()
    return _NC_CACHE


def kernel(**inputs) -> np.ndarray:
    nc = _program()
    maps = host_inputs(inputs)
    kwargs = {}
    if os.environ.get("BASSK_TRACE"):
        kwargs = dict(trace=True, trace_cores=list(range(NCORES)))
        td = os.environ.get("BASSK_TRACE_DIR")
        if td:
            kwargs["tmpdir"] = td
    res = bass_utils.run_bass_kernel_spmd(
        nc, maps, core_ids=list(range(NCORES)), **kwargs)
    kernel.last_results = res
    b_o = np.asarray(inputs["b_o"], np.float32)
    out = np.empty((B, S, MODEL), np.float32)
    for b in range(B):
        acc = np.asarray(res.results[b * 4]["OT"], np.float32)
        for c in range(b * 4 + 1, b * 4 + 4):
            acc = acc + np.asarray(res.results[c]["OT"], np.float32)
        out[b] = acc.T + b_o[None, :]
    out[:, :HOST_ROWS, :] = _host_head(inputs, HOST_ROWS)
    return out
